# revision 12
# baseline (speedup 1.0000x reference)
"""2-layer GraphConv GNN on 8 trn2 NeuronCores (Bass/Tile) — v4.

Design: aggregation entirely on the DMA stream (dma_gather + dma_scatter_add),
no per-edge compute instructions. ~700 instructions total.

  - Edges sharded by dst node (core c owns dst in [c*12500, (c+1)*12500)).
  - L1: gather x[src] rows (f32, 512B) from an AllGather-built table, then
    dma_scatter_add them into agg1[dst] (f32, DRAM). Same for L2 over the
    hr table (f32, 256B rows).
  - agg read back FEATURE-major in ONE transposed dma_gather (bf16, rows
    paired to satisfy the 256B elem minimum) -> dense 448/512-wide PE
    transforms, ACT relu+bias.
  - Internal node order sigma = [even nodes | odd nodes] so pair-stacked
    PSUM results transpose directly into natural node-major pair rows.
  - hr exchange: AllGather of per-core [12544, 64] f32 shards (padded to
    98*128); L2 gather indices account for the 12544 stride.
  - Upload: only bf16 x-shard + int16 idx pack (~5MB/core); output bf16.
"""

import numpy as np
import ml_dtypes
from contextlib import ExitStack

N = 100000
F = 128
O = 64
NC = 8
SHARD = N // NC          # 12500
SH2 = 12544              # padded shard rows (98*128) for hr/out
P2 = SH2 // 2            # 6272 sigma pair columns
PV = SHARD // 2          # 6250 valid pairs
PC = 6400                # padded pair count for transposed agg gathers
NR = 4
RS = N // NR             # 25000 (L1 gather ranges)
N2 = NC * SH2            # 100352 (hr_full rows)
RS2 = N2 // NR           # 25088 (L2 gather ranges)
CH = 5120                # rows per gather/scatter chunk

bf16 = ml_dtypes.bfloat16

# dynamic slot-layout profile, set by prepare_in_maps() before build:
# _PROF[L] = {"RB": range stride (mult of CH), "segs": [per-range list of
#             (start, len) color segments, 128-aligned]}
_PROF = {}


def _layout_consts():
    RB1, RB2 = _PROF[1]["RB"], _PROF[2]["RB"]
    TOT1, TOT2 = NR * RB1, NR * RB2
    OG1 = 0
    OS1 = OG1 + TOT1 // 16
    OG2 = OS1 + TOT1 // 16
    OS2 = OG2 + TOT2 // 16
    OXT = OS2 + TOT2 // 16
    OPX = OXT + SH2 // 16
    IDXW = OPX + PC // 16
    return RB1, RB2, TOT1, TOT2, OG1, OS1, OG2, OS2, OXT, OPX, IDXW

import os
_L1ONLY = bool(int(os.environ.get("GNN_L1ONLY", "0")))


def input_decls():
    IDXW = _layout_consts()[-1]
    return [
        ("xs", [SHARD, F], "bfloat16"),
        ("idxall", [16, IDXW], "int16"),
        ("wr1T", [F, F], "bfloat16"),
        ("wo1T", [F, F], "bfloat16"),
        ("wr2T", [F, O], "bfloat16"),
        ("wo2T", [F, O], "bfloat16"),
        ("b1c", [128, 1], "float32"),
        ("b2r", [1, O], "bfloat16"),
        ("ones", [1, 512], "bfloat16"),
        ("identb", [128, 128], "bfloat16"),
        ("ident32", [128, 128], "float32"),
    ]


def _build_program():
    import concourse.bass as bass
    import concourse.tile as tile
    from concourse import bacc, mybir

    RB1, RB2, TOT1, TOT2, OG1, OS1, OG2, OS2, OXT, OPX, IDXW = _layout_consts()
    nc = bacc.Bacc(None, target_bir_lowering=False, num_swdge_queues=4)
    dt = mybir.dt

    xs_in = nc.dram_tensor("xs", [SHARD, F], dt.bfloat16, kind="ExternalInput")
    idxall = nc.dram_tensor("idxall", [16, IDXW], dt.int16, kind="ExternalInput")
    wr1T = nc.dram_tensor("wr1T", [F, F], dt.bfloat16, kind="ExternalInput")
    wo1T = nc.dram_tensor("wo1T", [F, F], dt.bfloat16, kind="ExternalInput")
    wr2T = nc.dram_tensor("wr2T", [F, O], dt.bfloat16, kind="ExternalInput")
    wo2T = nc.dram_tensor("wo2T", [F, O], dt.bfloat16, kind="ExternalInput")
    b1c_in = nc.dram_tensor("b1c", [128, 1], dt.float32, kind="ExternalInput")
    b2r_in = nc.dram_tensor("b2r", [1, O], dt.bfloat16, kind="ExternalInput")
    ones_in = nc.dram_tensor("ones", [1, 512], dt.bfloat16, kind="ExternalInput")
    identb_in = nc.dram_tensor("identb", [128, 128], dt.bfloat16, kind="ExternalInput")
    ident32_in = nc.dram_tensor("ident32", [128, 128], dt.float32, kind="ExternalInput")
    out_t = nc.dram_tensor("out", [SH2, O], dt.bfloat16, kind="ExternalOutput")

    xs_int = nc.dram_tensor("xs_int", [SHARD, F], dt.bfloat16)
    xfull_bf = nc.dram_tensor("xfull_bf", [N, F], dt.bfloat16, addr_space="Shared")
    xfull32 = nc.dram_tensor("xfull32", [N, F], dt.float32)
    idxf = nc.dram_tensor("idxf", [128, IDXW], dt.int16)
    agg1 = nc.dram_tensor("agg1", [SHARD + 128, F], dt.float32)
    agg1b = nc.dram_tensor("agg1b", [SHARD + 128, F], dt.bfloat16)
    hr_shard = nc.dram_tensor("hr_shard", [SH2, O], dt.float32)
    hr_full = nc.dram_tensor("hr_full", [N2, O], dt.float32, addr_space="Shared")
    agg2 = nc.dram_tensor("agg2", [SHARD + 128, O], dt.float32)
    agg2b = nc.dram_tensor("agg2b", [SHARD + 128, O], dt.bfloat16)

    with tile.TileContext(nc) as tc, ExitStack() as ctx:
        const_p = ctx.enter_context(tc.tile_pool(name="const", bufs=1))
        resid_p = ctx.enter_context(tc.tile_pool(name="resid", bufs=1))
        idx_p = ctx.enter_context(tc.tile_pool(name="idxp", bufs=2))
        msgs_p = ctx.enter_context(tc.tile_pool(name="msgs", bufs=2))
        sb_p = ctx.enter_context(tc.tile_pool(name="sbp", bufs=2))
        ps_h = ctx.enter_context(tc.tile_pool(name="ps_h", bufs=2, space="PSUM"))
        ps_stk = ctx.enter_context(tc.tile_pool(name="ps_stk", bufs=2, space="PSUM"))
        ps_tr = ctx.enter_context(tc.tile_pool(name="ps_tr", bufs=2, space="PSUM"))
        ps_trb = ctx.enter_context(tc.tile_pool(name="ps_trb", bufs=2, space="PSUM"))

        # ---- prologue ----
        nc.sync.dma_start(xs_int[:], xs_in[:])
        nc.gpsimd.collective_compute(
            "AllGather",
            mybir.AluOpType.bypass,
            replica_groups=[list(range(NC))],
            ins=[xs_int[:]],
            outs=[xfull_bf[:]],
        )
        # cast-expand x table to f32 (flat [128, 100000])
        xb_flat = xfull_bf[:].rearrange("n f -> (n f)").rearrange("(a b) -> a b", a=128)
        x3_flat = xfull32[:].rearrange("n f -> (n f)").rearrange("(a b) -> a b", a=128)
        CW = xb_flat.shape[1]
        for i in range(4):
            lo = i * (CW // 4)
            hi = (i + 1) * (CW // 4) if i < 3 else CW
            nc.gpsimd.dma_start(x3_flat[:, lo:hi], xb_flat[:, lo:hi])
        # idx replication [16, W] -> [128, W]
        for k in range(8):
            nc.sync.dma_start(idxf[16 * k : 16 * (k + 1), :], idxall[:])

        c_wr1T = const_p.tile([F, F], dt.bfloat16)
        nc.sync.dma_start(c_wr1T[:], wr1T[:])
        c_wo1T = const_p.tile([F, F], dt.bfloat16)
        nc.sync.dma_start(c_wo1T[:], wo1T[:])
        c_wr2T = const_p.tile([F, O], dt.bfloat16)
        nc.sync.dma_start(c_wr2T[:], wr2T[:])
        c_wo2T = const_p.tile([F, O], dt.bfloat16)
        nc.sync.dma_start(c_wo2T[:], wo2T[:])
        c_b1c = const_p.tile([128, 1], dt.float32)
        nc.sync.dma_start(c_b1c[:], b1c_in[:])
        c_b2r = const_p.tile([1, O], dt.bfloat16)
        nc.sync.dma_start(c_b2r[:], b2r_in[:])
        c_ones = const_p.tile([1, 512], dt.bfloat16)
        nc.sync.dma_start(c_ones[:], ones_in[:])
        c_identb = const_p.tile([128, 128], dt.bfloat16)
        nc.sync.dma_start(c_identb[:], identb_in[:])
        c_ident32 = const_p.tile([128, 128], dt.float32)
        nc.sync.dma_start(c_ident32[:], ident32_in[:])

        # zero agg1 / agg2
        zt = const_p.tile([128, 2048], dt.float32)
        nc.vector.memset(zt[:], 0.0)
        a1_flat = agg1[:].rearrange("n f -> (n f)").rearrange("(a b) -> a b", a=128)
        W1 = a1_flat.shape[1]  # 12500
        for i in range(8):
            lo = i * 2048
            hi = min(W1, lo + 2048)
            if lo < W1:
                nc.sync.dma_start(a1_flat[:, lo:hi], zt[:, : hi - lo])
        a2_flat = agg2[:].rearrange("n f -> (n f)").rearrange("(a b) -> a b", a=128)
        W2 = a2_flat.shape[1]  # 6250
        for i in range(4):
            lo = i * 2048
            hi = min(W2, lo + 2048)
            if lo < W2:
                nc.sync.dma_start(a2_flat[:, lo:hi], zt[:, : hi - lo])

        # r_xiT: sigma-ordered feature-major x shard via one transposed gather
        r_xiT = resid_p.tile([128, SH2], dt.bfloat16)
        xt_idx = idx_p.tile([128, SH2 // 16], dt.int16, tag="bigidx")
        nc.sync.dma_start(xt_idx[:], idxf[:, OXT : OXT + SH2 // 16])
        nc.gpsimd.dma_gather(
            r_xiT[:].rearrange("p (c e) -> p c e", c=1),
            xs_int[:],
            xt_idx[:],
            SH2,
            SH2,
            F,
            transpose=True,
            single_packet=False,
            queue_num=0,
        )
        r_hT = resid_p.tile([128, SH2], dt.bfloat16)
        aggT1 = resid_p.tile([128, 2 * PC], dt.bfloat16)
        aggT2 = resid_p.tile([128, PC], dt.bfloat16)
        px_idx = idx_p.tile([128, PC // 16], dt.int16, tag="pidx")
        nc.sync.dma_start(px_idx[:], idxf[:, OPX : OPX + PC // 16])

        # resident scatter idx for current layer
        sidx_res = resid_p.tile([128, max(TOT1, TOT2) // 16], dt.int16)

        def gs_chunks(L):
            """Gather fixed chunks; scatter_add per (color segment x chunk)
            intersection so every scatter call has unique dst rows
            (dma_scatter_add loses adds on duplicate idx within a call)."""
            OG = OG1 if L == 1 else OG2
            OS = OS1 if L == 1 else OS2
            FW = F if L == 1 else O
            agg = agg1 if L == 1 else agg2
            table = xfull32 if L == 1 else hr_full
            RSL = RS if L == 1 else RS2
            RB = RB1 if L == 1 else RB2
            segs = _PROF[L]["segs"]
            NCH = RB // CH
            CHC = CH // 16
            nc.sync.dma_start(
                sidx_res[:, : (NR * RB) // 16], idxf[:, OS : OS + (NR * RB) // 16]
            )
            mtiles = {}
            for c in range(NR * NCH):
                r = c // NCH
                git = idx_p.tile([128, CHC], dt.int16, tag="git")
                nc.sync.dma_start(git[:], idxf[:, OG + c * CHC : OG + (c + 1) * CHC])
                m = msgs_p.tile([128, (CH // 128) * FW], dt.float32, tag="m")
                nc.gpsimd.dma_gather(
                    m[:].rearrange("p (c e) -> p c e", e=FW),
                    table[r * RSL : (r + 1) * RSL, :],
                    git[:],
                    CH,
                    CH,
                    FW,
                    single_packet=False,
                    queue_num=0,
                )
                mtiles[c] = m
                # scatter every (segment x this-chunk) intersection
                clo, chi = c * CH, (c + 1) * CH
                base = r * RB
                for (sst, sln) in segs[r]:
                    a = max(base + sst, clo)
                    b = min(base + sst + sln, chi)
                    if a >= b:
                        continue
                    off = a - clo  # 128-aligned
                    nrow = b - a
                    nc.gpsimd.dma_scatter_add(
                        agg[:],
                        m[:].rearrange("p (c e) -> p c e", e=FW)[
                            :, off // 128 : off // 128 + nrow // 128, :
                        ],
                        sidx_res[:, a // 16 : a // 16 + nrow // 16],
                        nrow,
                        nrow,
                        FW,
                        single_packet=False,
                        queue_num=0,
                    )

        # ================= layer 1 =================
        gs_chunks(1)
        # agg1 -> bf16
        a1b_flat = agg1b[:].rearrange("n f -> (n f)").rearrange("(a b) -> a b", a=128)
        for i in range(2):
            lo = i * (W1 // 2)
            hi = (i + 1) * (W1 // 2) if i < 1 else W1
            nc.gpsimd.dma_start(a1b_flat[:, lo:hi], a1_flat[:, lo:hi])
        # aggT1: [128, 2, PC] via transposed gather of paired rows (512B)
        nc.gpsimd.dma_gather(
            aggT1[:].rearrange("p (c e) -> p c e", c=2),
            agg1b[:].rearrange("(a b) f -> a (b f)", b=2),
            px_idx[:],
            PC,
            PC,
            2 * F,
            transpose=True,
            single_packet=False,
            queue_num=0,
        )
        # transform: h = relu(wr1@aggT + wo1@xT + b1), 28 batches of 448
        aggT1v = aggT1[:].rearrange("p (c e) -> p c e", c=2)
        for b in range(28):
            plane = b // 14
            lo = (b % 14) * 448
            ph = ps_h.tile([128, 512], dt.float32, tag="ph", space="PSUM")
            nc.tensor.matmul(
                ph[:, :448],
                lhsT=c_wr1T[:],
                rhs=aggT1v[:, plane, lo : lo + 448],
                start=True,
                stop=False,
            )
            nc.tensor.matmul(
                ph[:, :448],
                lhsT=c_wo1T[:],
                rhs=r_xiT[:, plane * P2 + lo : plane * P2 + lo + 448],
                start=False,
                stop=True,
            )
            nc.scalar.activation(
                out=r_hT[:, plane * P2 + lo : plane * P2 + lo + 448],
                in_=ph[:, :448],
                func=mybir.ActivationFunctionType.Relu,
                bias=c_b1c[:],
            )
        # hr = wr2 @ h, pair-stacked -> transpose -> node-major pair rows
        hr_pairs = hr_shard[:].rearrange("(q t) o -> q (t o)", t=2)  # [6272, 128]
        for b in range(13):
            lo = b * 512
            w = 512 if b < 12 else P2 - 12 * 512  # 128
            pstk = ps_stk.tile([128, 512], dt.float32, tag="stk", space="PSUM")
            nc.tensor.matmul(
                pstk[0:64, :w], lhsT=c_wr2T[:], rhs=r_hT[:, lo : lo + w],
                start=True, stop=True,
            )
            nc.tensor.matmul(
                pstk[64:128, :w], lhsT=c_wr2T[:], rhs=r_hT[:, P2 + lo : P2 + lo + w],
                start=True, stop=True,
            )
            stk_sb = sb_p.tile([128, 512], dt.float32, tag="stks")
            nc.scalar.copy(out=stk_sb[:, :w], in_=pstk[:, :w])
            ptr = ps_tr.tile([128, 512], dt.float32, tag="tr", space="PSUM")
            for t in range(w // 128):
                nc.tensor.transpose(
                    ptr[:, t * 128 : (t + 1) * 128],
                    stk_sb[:, t * 128 : (t + 1) * 128],
                    c_ident32[:],
                )
            hw_sb = sb_p.tile([128, 512], dt.float32, tag="hws")
            nc.scalar.copy(out=hw_sb[:, :w], in_=ptr[:, :w])
            nc.sync.dma_start(
                hr_pairs[lo : lo + w, :].rearrange("(c p) e -> p c e", p=128),
                hw_sb[:, :w].rearrange("p (c e) -> p c e", e=128),
            )

        if _L1ONLY:
            zo = sb_p.tile([128, O], dt.bfloat16, tag="zo")
            nc.vector.memset(zo[:], 0.0)
            nc.sync.dma_start(out_t[0:128, :], zo[:])
        else:
            # ================= exchange =================
            nc.gpsimd.collective_compute(
                "AllGather",
                mybir.AluOpType.bypass,
                replica_groups=[list(range(NC))],
                ins=[hr_shard[:]],
                outs=[hr_full[:]],
            )
            # ================= layer 2 =================
            gs_chunks(2)
            a2b_flat = agg2b[:].rearrange("n f -> (n f)").rearrange("(a b) -> a b", a=128)
            nc.gpsimd.dma_start(a2b_flat[:], a2_flat[:])
            # aggT2: stacked [128, PC] (paired 256B rows)
            nc.gpsimd.dma_gather(
                aggT2[:].rearrange("p (c e) -> p c e", c=1),
                agg2b[:].rearrange("(a b) f -> a (b f)", b=2),
                px_idx[:],
                PC,
                PC,
                2 * O,
                transpose=True,
                single_packet=False,
                queue_num=0,
            )
            # out = agg2 + wo2@h + b2, pair-stacked
            out_pairs = out_t[:].rearrange("(q t) o -> q (t o)", t=2)  # [6272, 128]
            for b in range(13):
                lo = b * 512
                w = 512 if b < 12 else P2 - 12 * 512
                pstk = ps_stk.tile([128, 512], dt.float32, tag="stk", space="PSUM")
                nc.tensor.matmul(
                    pstk[0:64, :w], lhsT=c_wo2T[:], rhs=r_hT[:, lo : lo + w],
                    start=True, stop=False,
                )
                nc.tensor.matmul(
                    pstk[0:64, :w], lhsT=c_b2r[:1, :], rhs=c_ones[:1, :w],
                    start=False, stop=True,
                )
                nc.tensor.matmul(
                    pstk[64:128, :w], lhsT=c_wo2T[:], rhs=r_hT[:, P2 + lo : P2 + lo + w],
                    start=True, stop=False,
                )
                nc.tensor.matmul(
                    pstk[64:128, :w], lhsT=c_b2r[:1, :], rhs=c_ones[:1, :w],
                    start=False, stop=True,
                )
                stk_sb = sb_p.tile([128, 512], dt.bfloat16, tag="stk2s")
                nc.vector.tensor_tensor(
                    out=stk_sb[:, :w],
                    in0=pstk[:, :w],
                    in1=aggT2[:, lo : lo + w],
                    op=mybir.AluOpType.add,
                )
                ptrb = ps_trb.tile([128, 512], dt.bfloat16, tag="trb", space="PSUM")
                for t in range(w // 128):
                    nc.tensor.transpose(
                        ptrb[:, t * 128 : (t + 1) * 128],
                        stk_sb[:, t * 128 : (t + 1) * 128],
                        c_identb[:],
                    )
                ob_sb = sb_p.tile([128, 512], dt.bfloat16, tag="obs")
                nc.scalar.copy(out=ob_sb[:, :w], in_=ptrb[:, :w])
                nc.sync.dma_start(
                    out_pairs[lo : lo + w, :].rearrange("(c p) e -> p c e", p=128),
                    ob_sb[:, :w].rearrange("p (c e) -> p c e", e=128),
                )

    nc.finalize()
    return nc


_CACHED = {}


def _wrap16(flat):
    """[K] int16 -> [16, K//16], slot i at (i%16, i//16)."""
    return np.ascontiguousarray(flat.reshape(-1, 16).T)


def _rup(x, m):
    return (x + m - 1) // m * m


def _occ_split(gv, d, rr):
    """Per range: dst-stable order + occurrence index (k-th edge of its dst)."""
    out = []
    for r in range(NR):
        sel = rr == r
        gs, ds = gv[sel], d[sel]
        order = np.argsort(ds, kind="stable")
        ds_o, gs_o = ds[order], gs[order]
        cnt = np.bincount(ds_o, minlength=SHARD)
        start = np.zeros(SHARD + 1, dtype=np.int64)
        np.cumsum(cnt, out=start[1:])
        occ = np.arange(len(ds_o)) - start[ds_o]
        out.append((gs_o, ds_o, occ))
    return out


def _build_profile(all_splits):
    """all_splits: per core list of per-range (gs, ds, occ). Returns
    {"RB": int, "segs": [[(start, len)] per range]} with 128-aligned
    color-segment budgets = max over cores + margin."""
    maxk = 0
    for splits in all_splits:
        for (_, _, occ) in splits:
            if len(occ):
                maxk = max(maxk, int(occ.max()) + 1)
    sizes = np.zeros((NR, maxk), dtype=np.int64)
    for splits in all_splits:
        for r, (_, _, occ) in enumerate(splits):
            c = np.bincount(occ, minlength=maxk)
            sizes[r] = np.maximum(sizes[r], c[:maxk])
    budgets = np.vectorize(lambda s: _rup(s + 128, 128))(sizes)
    segs = []
    for r in range(NR):
        st = np.zeros(maxk + 1, dtype=np.int64)
        np.cumsum(budgets[r], out=st[1:])
        segs.append([(int(st[k]), int(budgets[r][k])) for k in range(maxk)])
    RB = _rup(int(max(np.sum(budgets[r]) for r in range(NR))), CH)
    return {"RB": RB, "segs": segs}


def _fill_layout(splits, prof):
    """Returns (g [TOT], s [TOT]) int16; gather pad idx 0, scatter pad -1."""
    RB = prof["RB"]
    segs = prof["segs"]
    TOT = NR * RB
    g = np.zeros(TOT, dtype=np.int16)
    s = np.full(TOT, SHARD, dtype=np.int16)  # pad -> dump row (races harmless)
    for r, (gs_o, ds_o, occ) in enumerate(splits):
        o2 = np.argsort(occ, kind="stable")
        occ_s = occ[o2]
        nk = int(occ_s.max()) + 1 if len(occ_s) else 0
        kstart = np.searchsorted(occ_s, np.arange(nk + 1))
        within = np.arange(len(occ_s)) - kstart[occ_s]
        segstart = np.array([segs[r][k][0] for k in range(nk)], dtype=np.int64)
        seglen = np.array([segs[r][k][1] for k in range(nk)], dtype=np.int64)
        if np.any(np.bincount(occ_s, minlength=nk)[:nk] > seglen):
            raise RuntimeError("color segment overflow")
        slot = r * RB + segstart[occ_s] + within
        g[slot] = gs_o[o2].astype(np.int16)
        s[slot] = ds_o[o2].astype(np.int16)
    return g, s


def prepare_in_maps(inputs):
    x = np.asarray(inputs["x"], dtype=np.float32)
    edge_index = np.asarray(inputs["edge_index"])
    w_rel1 = np.asarray(inputs["w_rel1"], dtype=np.float32)
    b_rel1 = np.asarray(inputs["b_rel1"], dtype=np.float32)
    w_root1 = np.asarray(inputs["w_root1"], dtype=np.float32)
    w_rel2 = np.asarray(inputs["w_rel2"], dtype=np.float32)
    b_rel2 = np.asarray(inputs["b_rel2"], dtype=np.float32)
    w_root2 = np.asarray(inputs["w_root2"], dtype=np.float32)

    src = edge_index[0].astype(np.int64)
    dst = edge_index[1].astype(np.int64)
    xbf = x.astype(bf16)

    # sigma-ordered xiT gather idx + pair idx (same for all cores except xiT)
    xt = np.zeros(SH2, dtype=np.int16)
    xt[0:PV] = (np.arange(PV) * 2).astype(np.int16)
    xt[P2 : P2 + PV] = (np.arange(PV) * 2 + 1).astype(np.int16)
    px = np.zeros(PC, dtype=np.int16)
    px[0:PV] = np.arange(PV, dtype=np.int16)

    identb = np.eye(128, dtype=np.float32).astype(bf16)
    ident32 = np.eye(128, dtype=np.float32)
    b1c = np.zeros((128, 1), np.float32)
    b1c[: len(b_rel1), 0] = b_rel1

    core = dst // SHARD
    order = np.argsort(core, kind="stable")
    src_s, dst_s = src[order], dst[order]
    bounds = np.searchsorted(core[order], np.arange(NC + 1))

    # pass 1: per-core per-range occurrence splits for both layers
    splits1, splits2 = [], []
    for c in range(NC):
        lo, hi = bounds[c], bounds[c + 1]
        sc, dc = src_s[lo:hi], dst_s[lo:hi] - c * SHARD
        splits1.append(_occ_split(sc % RS, dc, sc // RS))
        gsrc2 = (sc // SHARD) * SH2 + (sc % SHARD)
        splits2.append(_occ_split(gsrc2 % RS2, dc, gsrc2 // RS2))
    _PROF[1] = _build_profile(splits1)
    _PROF[2] = _build_profile(splits2)
    IDXW = _layout_consts()[-1]

    in_maps = []
    for c in range(NC):
        g1, s1 = _fill_layout(splits1[c], _PROF[1])
        g2, s2 = _fill_layout(splits2[c], _PROF[2])
        idx_pack = np.concatenate(
            [
                _wrap16(g1), _wrap16(s1), _wrap16(g2), _wrap16(s2),
                _wrap16(xt), _wrap16(px),
            ],
            axis=1,
        )
        assert idx_pack.shape == (16, IDXW)
        in_maps.append(
            {
                "xs": xbf[c * SHARD : (c + 1) * SHARD, :],
                "idxall": idx_pack,
                "wr1T": np.ascontiguousarray(w_rel1.T).astype(bf16),
                "wo1T": np.ascontiguousarray(w_root1.T).astype(bf16),
                "wr2T": np.ascontiguousarray(w_rel2.T).astype(bf16),
                "wo2T": np.ascontiguousarray(w_root2.T).astype(bf16),
                "b1c": b1c,
                "b2r": b_rel2.reshape(1, O).astype(bf16),
                "ones": np.ones((1, 512), np.float32).astype(bf16),
                "identb": identb,
                "ident32": ident32,
            }
        )
    return in_maps


def get_nc():
    if "nc" not in _CACHED:
        _CACHED["nc"] = _build_program()
    return _CACHED["nc"]


def kernel(**inputs):
    from concourse.bass_utils import run_bass_kernel_spmd

    in_maps = prepare_in_maps(inputs)
    nc = get_nc()
    res = run_bass_kernel_spmd(nc, in_maps, core_ids=list(range(NC)), trace=False)
    out = np.concatenate(
        [res.results[c]["out"][:SHARD] for c in range(NC)], axis=0
    )
    return out.astype(np.float32)


# revision 13
# speedup vs baseline: 1.0737x; 1.0737x over previous
"""2-layer GraphConv GNN on 8 trn2 NeuronCores (Bass/Tile) — v4.

Design: aggregation entirely on the DMA stream (dma_gather + dma_scatter_add),
no per-edge compute instructions. ~700 instructions total.

  - Edges sharded by dst node (core c owns dst in [c*12500, (c+1)*12500)).
  - L1: gather x[src] rows (f32, 512B) from an AllGather-built table, then
    dma_scatter_add them into agg1[dst] (f32, DRAM). Same for L2 over the
    hr table (f32, 256B rows).
  - agg read back FEATURE-major in ONE transposed dma_gather (bf16, rows
    paired to satisfy the 256B elem minimum) -> dense 448/512-wide PE
    transforms, ACT relu+bias.
  - Internal node order sigma = [even nodes | odd nodes] so pair-stacked
    PSUM results transpose directly into natural node-major pair rows.
  - hr exchange: AllGather of per-core [12544, 64] f32 shards (padded to
    98*128); L2 gather indices account for the 12544 stride.
  - Upload: only bf16 x-shard + int16 idx pack (~5MB/core); output bf16.
"""

import numpy as np
import ml_dtypes
from contextlib import ExitStack

N = 100000
F = 128
O = 64
NC = 8
SHARD = N // NC          # 12500
SH2 = 12544              # padded shard rows (98*128) for hr/out
P2 = SH2 // 2            # 6272 sigma pair columns
PV = SHARD // 2          # 6250 valid pairs
PC = 6400                # padded pair count for transposed agg gathers
NR = 4
RS = N // NR             # 25000 (L1 gather ranges)
N2 = NC * SH2            # 100352 (hr_full rows)
RS2 = N2 // NR           # 25088 (L2 gather ranges)
CH = 5120                # rows per gather/scatter chunk

bf16 = ml_dtypes.bfloat16

# dynamic slot-layout profile, set by prepare_in_maps() before build:
# _PROF[L] = {"RB": range stride (mult of CH), "segs": [per-range list of
#             (start, len) color segments, 128-aligned]}
_PROF = {}


def _layout_consts():
    RB1, RB2 = _PROF[1]["RB"], _PROF[2]["RB"]
    TOT1, TOT2 = NR * RB1, NR * RB2
    OG1 = 0
    OS1 = OG1 + TOT1 // 16
    OG2 = OS1 + TOT1 // 16
    OS2 = OG2 + TOT2 // 16
    OXT = OS2 + TOT2 // 16
    OPX = OXT + SH2 // 16
    IDXW = OPX + PC // 16
    return RB1, RB2, TOT1, TOT2, OG1, OS1, OG2, OS2, OXT, OPX, IDXW

import os
_L1ONLY = bool(int(os.environ.get("GNN_L1ONLY", "0")))


def input_decls():
    IDXW = _layout_consts()[-1]
    return [
        ("xs", [SHARD, F], "bfloat16"),
        ("idxall", [16, IDXW], "int16"),
        ("wr1T", [F, F], "bfloat16"),
        ("wo1T", [F, F], "bfloat16"),
        ("wr2T", [F, O], "bfloat16"),
        ("wo2T", [F, O], "bfloat16"),
        ("b1c", [128, 1], "float32"),
        ("b2r", [1, O], "bfloat16"),
        ("ones", [1, 512], "bfloat16"),
        ("identb", [128, 128], "bfloat16"),
        ("ident32", [128, 128], "float32"),
    ]


def _build_program():
    import concourse.bass as bass
    import concourse.tile as tile
    from concourse import bacc, mybir

    RB1, RB2, TOT1, TOT2, OG1, OS1, OG2, OS2, OXT, OPX, IDXW = _layout_consts()
    nc = bacc.Bacc(None, target_bir_lowering=False, num_swdge_queues=4)
    dt = mybir.dt

    xs_in = nc.dram_tensor("xs", [SHARD, F], dt.bfloat16, kind="ExternalInput")
    idxall = nc.dram_tensor("idxall", [16, IDXW], dt.int16, kind="ExternalInput")
    wr1T = nc.dram_tensor("wr1T", [F, F], dt.bfloat16, kind="ExternalInput")
    wo1T = nc.dram_tensor("wo1T", [F, F], dt.bfloat16, kind="ExternalInput")
    wr2T = nc.dram_tensor("wr2T", [F, O], dt.bfloat16, kind="ExternalInput")
    wo2T = nc.dram_tensor("wo2T", [F, O], dt.bfloat16, kind="ExternalInput")
    b1c_in = nc.dram_tensor("b1c", [128, 1], dt.float32, kind="ExternalInput")
    b2r_in = nc.dram_tensor("b2r", [1, O], dt.bfloat16, kind="ExternalInput")
    ones_in = nc.dram_tensor("ones", [1, 512], dt.bfloat16, kind="ExternalInput")
    identb_in = nc.dram_tensor("identb", [128, 128], dt.bfloat16, kind="ExternalInput")
    ident32_in = nc.dram_tensor("ident32", [128, 128], dt.float32, kind="ExternalInput")
    out_t = nc.dram_tensor("out", [SH2, O], dt.bfloat16, kind="ExternalOutput")

    xs_int = nc.dram_tensor("xs_int", [SHARD, F], dt.bfloat16)
    xfull_bf = nc.dram_tensor("xfull_bf", [N, F], dt.bfloat16, addr_space="Shared")
    xfull32 = nc.dram_tensor("xfull32", [N, F], dt.float32)
    idxf = nc.dram_tensor("idxf", [128, IDXW], dt.int16)
    agg1 = nc.dram_tensor("agg1", [SHARD + 128, F], dt.float32)
    agg1b = nc.dram_tensor("agg1b", [SHARD + 128, F], dt.bfloat16)
    hr_shard = nc.dram_tensor("hr_shard", [SH2, O], dt.float32)
    hr_full = nc.dram_tensor("hr_full", [N2, O], dt.float32, addr_space="Shared")
    agg2 = nc.dram_tensor("agg2", [SHARD + 128, O], dt.float32)
    agg2b = nc.dram_tensor("agg2b", [SHARD + 128, O], dt.bfloat16)

    with tile.TileContext(nc) as tc, ExitStack() as ctx:
        const_p = ctx.enter_context(tc.tile_pool(name="const", bufs=1))
        resid_p = ctx.enter_context(tc.tile_pool(name="resid", bufs=1))
        idx_p = ctx.enter_context(tc.tile_pool(name="idxp", bufs=2))
        msgs_p = ctx.enter_context(tc.tile_pool(name="msgs", bufs=2))
        sb_p = ctx.enter_context(tc.tile_pool(name="sbp", bufs=2))
        ps_h = ctx.enter_context(tc.tile_pool(name="ps_h", bufs=2, space="PSUM"))
        ps_stk = ctx.enter_context(tc.tile_pool(name="ps_stk", bufs=2, space="PSUM"))
        ps_tr = ctx.enter_context(tc.tile_pool(name="ps_tr", bufs=2, space="PSUM"))
        ps_trb = ctx.enter_context(tc.tile_pool(name="ps_trb", bufs=2, space="PSUM"))

        # ---- prologue ----
        nc.sync.dma_start(xs_int[:], xs_in[:])
        nc.gpsimd.collective_compute(
            "AllGather",
            mybir.AluOpType.bypass,
            replica_groups=[list(range(NC))],
            ins=[xs_int[:]],
            outs=[xfull_bf[:]],
        )
        # cast-expand x table to f32 (flat [128, 100000])
        xb_flat = xfull_bf[:].rearrange("n f -> (n f)").rearrange("(a b) -> a b", a=128)
        x3_flat = xfull32[:].rearrange("n f -> (n f)").rearrange("(a b) -> a b", a=128)
        CW = xb_flat.shape[1]
        for i in range(4):
            lo = i * (CW // 4)
            hi = (i + 1) * (CW // 4) if i < 3 else CW
            nc.gpsimd.dma_start(x3_flat[:, lo:hi], xb_flat[:, lo:hi])
        # idx replication [16, W] -> [128, W]
        for k in range(8):
            nc.sync.dma_start(idxf[16 * k : 16 * (k + 1), :], idxall[:])

        c_wr1T = const_p.tile([F, F], dt.bfloat16)
        nc.sync.dma_start(c_wr1T[:], wr1T[:])
        c_wo1T = const_p.tile([F, F], dt.bfloat16)
        nc.sync.dma_start(c_wo1T[:], wo1T[:])
        c_wr2T = const_p.tile([F, O], dt.bfloat16)
        nc.sync.dma_start(c_wr2T[:], wr2T[:])
        c_wo2T = const_p.tile([F, O], dt.bfloat16)
        nc.sync.dma_start(c_wo2T[:], wo2T[:])
        c_b1c = const_p.tile([128, 1], dt.float32)
        nc.sync.dma_start(c_b1c[:], b1c_in[:])
        c_b2r = const_p.tile([1, O], dt.bfloat16)
        nc.sync.dma_start(c_b2r[:], b2r_in[:])
        c_ones = const_p.tile([1, 512], dt.bfloat16)
        nc.sync.dma_start(c_ones[:], ones_in[:])
        c_identb = const_p.tile([128, 128], dt.bfloat16)
        nc.sync.dma_start(c_identb[:], identb_in[:])
        c_ident32 = const_p.tile([128, 128], dt.float32)
        nc.sync.dma_start(c_ident32[:], ident32_in[:])

        # zero agg1 / agg2
        zt = const_p.tile([128, 2048], dt.float32)
        nc.vector.memset(zt[:], 0.0)
        a1_flat = agg1[:].rearrange("n f -> (n f)").rearrange("(a b) -> a b", a=128)
        W1 = a1_flat.shape[1]  # 12500
        for i in range(8):
            lo = i * 2048
            hi = min(W1, lo + 2048)
            if lo < W1:
                nc.sync.dma_start(a1_flat[:, lo:hi], zt[:, : hi - lo])
        a2_flat = agg2[:].rearrange("n f -> (n f)").rearrange("(a b) -> a b", a=128)
        W2 = a2_flat.shape[1]  # 6250
        for i in range(4):
            lo = i * 2048
            hi = min(W2, lo + 2048)
            if lo < W2:
                nc.sync.dma_start(a2_flat[:, lo:hi], zt[:, : hi - lo])

        # r_xiT: sigma-ordered feature-major x shard via one transposed gather
        r_xiT = resid_p.tile([128, SH2], dt.bfloat16)
        xt_idx = idx_p.tile([128, SH2 // 16], dt.int16, tag="bigidx")
        nc.sync.dma_start(xt_idx[:], idxf[:, OXT : OXT + SH2 // 16])
        nc.gpsimd.dma_gather(
            r_xiT[:].rearrange("p (c e) -> p c e", c=1),
            xs_int[:],
            xt_idx[:],
            SH2,
            SH2,
            F,
            transpose=True,
            single_packet=False,
            queue_num=0,
        )
        r_hT = resid_p.tile([128, SH2], dt.bfloat16)
        aggT1 = resid_p.tile([128, 2 * PC], dt.bfloat16)
        aggT2 = resid_p.tile([128, PC], dt.bfloat16)
        px_idx = idx_p.tile([128, PC // 16], dt.int16, tag="pidx")
        nc.sync.dma_start(px_idx[:], idxf[:, OPX : OPX + PC // 16])

        # resident scatter idx for current layer
        sidx_res = resid_p.tile([128, max(TOT1, TOT2) // 16], dt.int16)

        def gs_chunks(L):
            """Gather fixed chunks; scatter_add per (color segment x chunk)
            intersection so every scatter call has unique dst rows
            (dma_scatter_add loses adds on duplicate idx within a call)."""
            OG = OG1 if L == 1 else OG2
            OS = OS1 if L == 1 else OS2
            FW = F if L == 1 else O
            agg = agg1 if L == 1 else agg2
            table = xfull32 if L == 1 else hr_full
            RSL = RS if L == 1 else RS2
            RB = RB1 if L == 1 else RB2
            segs = _PROF[L]["segs"]
            NCH = RB // CH
            CHC = CH // 16
            nc.sync.dma_start(
                sidx_res[:, : (NR * RB) // 16], idxf[:, OS : OS + (NR * RB) // 16]
            )
            mtiles = {}
            git = None
            for c in range(NR * NCH):
                r = c // NCH
                if c % NCH == 0:
                    git = idx_p.tile([128, RB // 16], dt.int16, tag="git")
                    nc.sync.dma_start(
                        git[:], idxf[:, OG + r * (RB // 16) : OG + (r + 1) * (RB // 16)]
                    )
                k = c % NCH
                m = msgs_p.tile([128, (CH // 128) * FW], dt.float32, tag="m")
                nc.gpsimd.dma_gather(
                    m[:].rearrange("p (c e) -> p c e", e=FW),
                    table[r * RSL : (r + 1) * RSL, :],
                    git[:, k * CHC : (k + 1) * CHC],
                    CH,
                    CH,
                    FW,
                    single_packet=False,
                    queue_num=0,
                )
                mtiles[c] = m
                # scatter every (segment x this-chunk) intersection
                clo, chi = c * CH, (c + 1) * CH
                base = r * RB
                for (sst, sln) in segs[r]:
                    a = max(base + sst, clo)
                    b = min(base + sst + sln, chi)
                    if a >= b:
                        continue
                    off = a - clo  # 128-aligned
                    nrow = b - a
                    nc.gpsimd.dma_scatter_add(
                        agg[:],
                        m[:].rearrange("p (c e) -> p c e", e=FW)[
                            :, off // 128 : off // 128 + nrow // 128, :
                        ],
                        sidx_res[:, a // 16 : a // 16 + nrow // 16],
                        nrow,
                        nrow,
                        FW,
                        single_packet=False,
                        queue_num=0,
                    )

        # ================= layer 1 =================
        gs_chunks(1)
        # agg1 -> bf16
        a1b_flat = agg1b[:].rearrange("n f -> (n f)").rearrange("(a b) -> a b", a=128)
        for i in range(2):
            lo = i * (W1 // 2)
            hi = (i + 1) * (W1 // 2) if i < 1 else W1
            nc.gpsimd.dma_start(a1b_flat[:, lo:hi], a1_flat[:, lo:hi])
        # aggT1: [128, 2, PC] via transposed gather of paired rows (512B)
        nc.gpsimd.dma_gather(
            aggT1[:].rearrange("p (c e) -> p c e", c=2),
            agg1b[:].rearrange("(a b) f -> a (b f)", b=2),
            px_idx[:],
            PC,
            PC,
            2 * F,
            transpose=True,
            single_packet=False,
            queue_num=0,
        )
        # transform: h = relu(wr1@aggT + wo1@xT + b1), 28 batches of 448
        aggT1v = aggT1[:].rearrange("p (c e) -> p c e", c=2)
        for b in range(28):
            plane = b // 14
            lo = (b % 14) * 448
            ph = ps_h.tile([128, 512], dt.float32, tag="ph", space="PSUM")
            nc.tensor.matmul(
                ph[:, :448],
                lhsT=c_wr1T[:],
                rhs=aggT1v[:, plane, lo : lo + 448],
                start=True,
                stop=False,
            )
            nc.tensor.matmul(
                ph[:, :448],
                lhsT=c_wo1T[:],
                rhs=r_xiT[:, plane * P2 + lo : plane * P2 + lo + 448],
                start=False,
                stop=True,
            )
            nc.scalar.activation(
                out=r_hT[:, plane * P2 + lo : plane * P2 + lo + 448],
                in_=ph[:, :448],
                func=mybir.ActivationFunctionType.Relu,
                bias=c_b1c[:],
            )
        # hr = wr2 @ h, pair-stacked -> transpose -> node-major pair rows
        hr_pairs = hr_shard[:].rearrange("(q t) o -> q (t o)", t=2)  # [6272, 128]
        for b in range(13):
            lo = b * 512
            w = 512 if b < 12 else P2 - 12 * 512  # 128
            pstk = ps_stk.tile([128, 512], dt.float32, tag="stk", space="PSUM")
            nc.tensor.matmul(
                pstk[0:64, :w], lhsT=c_wr2T[:], rhs=r_hT[:, lo : lo + w],
                start=True, stop=True,
            )
            nc.tensor.matmul(
                pstk[64:128, :w], lhsT=c_wr2T[:], rhs=r_hT[:, P2 + lo : P2 + lo + w],
                start=True, stop=True,
            )
            stk_sb = sb_p.tile([128, 512], dt.float32, tag="stks")
            nc.scalar.copy(out=stk_sb[:, :w], in_=pstk[:, :w])
            ptr = ps_tr.tile([128, 512], dt.float32, tag="tr", space="PSUM")
            for t in range(w // 128):
                nc.tensor.transpose(
                    ptr[:, t * 128 : (t + 1) * 128],
                    stk_sb[:, t * 128 : (t + 1) * 128],
                    c_ident32[:],
                )
            hw_sb = sb_p.tile([128, 512], dt.float32, tag="hws")
            nc.scalar.copy(out=hw_sb[:, :w], in_=ptr[:, :w])
            nc.sync.dma_start(
                hr_pairs[lo : lo + w, :].rearrange("(c p) e -> p c e", p=128),
                hw_sb[:, :w].rearrange("p (c e) -> p c e", e=128),
            )

        if _L1ONLY:
            zo = sb_p.tile([128, O], dt.bfloat16, tag="zo")
            nc.vector.memset(zo[:], 0.0)
            nc.sync.dma_start(out_t[0:128, :], zo[:])
        else:
            # ================= exchange =================
            nc.gpsimd.collective_compute(
                "AllGather",
                mybir.AluOpType.bypass,
                replica_groups=[list(range(NC))],
                ins=[hr_shard[:]],
                outs=[hr_full[:]],
            )
            # ================= layer 2 =================
            gs_chunks(2)
            a2b_flat = agg2b[:].rearrange("n f -> (n f)").rearrange("(a b) -> a b", a=128)
            nc.gpsimd.dma_start(a2b_flat[:], a2_flat[:])
            # aggT2: stacked [128, PC] (paired 256B rows)
            nc.gpsimd.dma_gather(
                aggT2[:].rearrange("p (c e) -> p c e", c=1),
                agg2b[:].rearrange("(a b) f -> a (b f)", b=2),
                px_idx[:],
                PC,
                PC,
                2 * O,
                transpose=True,
                single_packet=False,
                queue_num=0,
            )
            # out = agg2 + wo2@h + b2, pair-stacked
            out_pairs = out_t[:].rearrange("(q t) o -> q (t o)", t=2)  # [6272, 128]
            for b in range(13):
                lo = b * 512
                w = 512 if b < 12 else P2 - 12 * 512
                pstk = ps_stk.tile([128, 512], dt.float32, tag="stk", space="PSUM")
                nc.tensor.matmul(
                    pstk[0:64, :w], lhsT=c_wo2T[:], rhs=r_hT[:, lo : lo + w],
                    start=True, stop=False,
                )
                nc.tensor.matmul(
                    pstk[0:64, :w], lhsT=c_b2r[:1, :], rhs=c_ones[:1, :w],
                    start=False, stop=True,
                )
                nc.tensor.matmul(
                    pstk[64:128, :w], lhsT=c_wo2T[:], rhs=r_hT[:, P2 + lo : P2 + lo + w],
                    start=True, stop=False,
                )
                nc.tensor.matmul(
                    pstk[64:128, :w], lhsT=c_b2r[:1, :], rhs=c_ones[:1, :w],
                    start=False, stop=True,
                )
                stk_sb = sb_p.tile([128, 512], dt.bfloat16, tag="stk2s")
                nc.vector.tensor_tensor(
                    out=stk_sb[:, :w],
                    in0=pstk[:, :w],
                    in1=aggT2[:, lo : lo + w],
                    op=mybir.AluOpType.add,
                )
                ptrb = ps_trb.tile([128, 512], dt.bfloat16, tag="trb", space="PSUM")
                for t in range(w // 128):
                    nc.tensor.transpose(
                        ptrb[:, t * 128 : (t + 1) * 128],
                        stk_sb[:, t * 128 : (t + 1) * 128],
                        c_identb[:],
                    )
                ob_sb = sb_p.tile([128, 512], dt.bfloat16, tag="obs")
                nc.scalar.copy(out=ob_sb[:, :w], in_=ptrb[:, :w])
                nc.sync.dma_start(
                    out_pairs[lo : lo + w, :].rearrange("(c p) e -> p c e", p=128),
                    ob_sb[:, :w].rearrange("p (c e) -> p c e", e=128),
                )

    nc.finalize()
    return nc


_CACHED = {}


def _wrap16(flat):
    """[K] int16 -> [16, K//16], slot i at (i%16, i//16)."""
    return np.ascontiguousarray(flat.reshape(-1, 16).T)


def _rup(x, m):
    return (x + m - 1) // m * m


def _occ_split(gv, d, rr):
    """(r, dst)-stable order + per-(r,dst) occurrence index, one global sort."""
    keyA = (rr * SHARD + d).astype(np.int32)
    oA = np.argsort(keyA, kind="stable")
    g_o, d_o, r_o = gv[oA], d[oA], rr[oA]
    grp = keyA[oA]
    cnt = np.bincount(grp, minlength=NR * SHARD)
    st = np.zeros(NR * SHARD + 1, dtype=np.int64)
    np.cumsum(cnt, out=st[1:])
    occ = np.arange(len(grp)) - st[grp]
    return g_o, d_o, r_o, occ


def _build_profile(all_splits):
    """all_splits: per core (g_o, d_o, r_o, occ). Returns {"RB", "segs"} with
    128-aligned color-segment budgets = max over cores + margin."""
    maxk = max(
        (int(occ.max()) + 1 if len(occ) else 0) for (_, _, _, occ) in all_splits
    )
    sizes = np.zeros((NR, maxk), dtype=np.int64)
    for (_, _, r_o, occ) in all_splits:
        c = np.bincount(r_o * maxk + occ, minlength=NR * maxk).reshape(NR, maxk)
        sizes = np.maximum(sizes, c)
    budgets = (sizes + 128 + 127) // 128 * 128
    segs = []
    for r in range(NR):
        st = np.zeros(maxk + 1, dtype=np.int64)
        np.cumsum(budgets[r], out=st[1:])
        segs.append([(int(st[k]), int(budgets[r][k])) for k in range(maxk)])
    RB = _rup(int(max(np.sum(budgets[r]) for r in range(NR))), CH)
    return {"RB": RB, "segs": segs}


def _fill_layout(split, prof):
    """Returns (g [TOT], s [TOT]) int16; gather pad idx 0, scatter pad dump."""
    RB = prof["RB"]
    segs = prof["segs"]
    MK = len(segs[0])
    TOT = NR * RB
    g_o, d_o, r_o, occ = split
    keyB = (r_o * MK + occ).astype(np.int32)
    oB = np.argsort(keyB, kind="stable")
    kB = keyB[oB]
    cnt = np.bincount(kB, minlength=NR * MK)
    st = np.zeros(NR * MK + 1, dtype=np.int64)
    np.cumsum(cnt, out=st[1:])
    pos = np.arange(len(kB)) - st[kB]
    segstart = np.array(
        [segs[r][k][0] for r in range(NR) for k in range(MK)], dtype=np.int64
    )
    seglen = np.array(
        [segs[r][k][1] for r in range(NR) for k in range(MK)], dtype=np.int64
    )
    if np.any(cnt > seglen):
        raise RuntimeError("color segment overflow")
    slot = r_o[oB] * RB + segstart[kB] + pos
    g = np.zeros(TOT, dtype=np.int16)
    s = np.full(TOT, SHARD, dtype=np.int16)  # pad -> dump row (races harmless)
    g[slot] = g_o[oB].astype(np.int16)
    s[slot] = d_o[oB].astype(np.int16)
    return g, s


def prepare_in_maps(inputs):
    x = np.asarray(inputs["x"], dtype=np.float32)
    edge_index = np.asarray(inputs["edge_index"])
    w_rel1 = np.asarray(inputs["w_rel1"], dtype=np.float32)
    b_rel1 = np.asarray(inputs["b_rel1"], dtype=np.float32)
    w_root1 = np.asarray(inputs["w_root1"], dtype=np.float32)
    w_rel2 = np.asarray(inputs["w_rel2"], dtype=np.float32)
    b_rel2 = np.asarray(inputs["b_rel2"], dtype=np.float32)
    w_root2 = np.asarray(inputs["w_root2"], dtype=np.float32)

    src = edge_index[0].astype(np.int32)
    dst = edge_index[1].astype(np.int32)
    xbf = x.astype(bf16)

    # sigma-ordered xiT gather idx + pair idx (same for all cores except xiT)
    xt = np.zeros(SH2, dtype=np.int16)
    xt[0:PV] = (np.arange(PV) * 2).astype(np.int16)
    xt[P2 : P2 + PV] = (np.arange(PV) * 2 + 1).astype(np.int16)
    px = np.zeros(PC, dtype=np.int16)
    px[0:PV] = np.arange(PV, dtype=np.int16)

    identb = np.eye(128, dtype=np.float32).astype(bf16)
    ident32 = np.eye(128, dtype=np.float32)
    b1c = np.zeros((128, 1), np.float32)
    b1c[: len(b_rel1), 0] = b_rel1

    core = (dst // SHARD).astype(np.int32)
    order = np.argsort(core, kind="stable")
    src_s, dst_s = src[order], dst[order]
    bounds = np.searchsorted(core[order], np.arange(NC + 1))

    # pass 1: per-core per-range occurrence splits for both layers
    splits1, splits2 = [], []
    for c in range(NC):
        lo, hi = bounds[c], bounds[c + 1]
        sc, dc = src_s[lo:hi], dst_s[lo:hi] - c * SHARD
        splits1.append(_occ_split(sc % RS, dc, sc // RS))
        gsrc2 = (sc // SHARD) * SH2 + (sc % SHARD)
        splits2.append(_occ_split(gsrc2 % RS2, dc, gsrc2 // RS2))
    _PROF[1] = _build_profile(splits1)
    _PROF[2] = _build_profile(splits2)
    IDXW = _layout_consts()[-1]

    in_maps = []
    for c in range(NC):
        g1, s1 = _fill_layout(splits1[c], _PROF[1])
        g2, s2 = _fill_layout(splits2[c], _PROF[2])
        idx_pack = np.concatenate(
            [
                _wrap16(g1), _wrap16(s1), _wrap16(g2), _wrap16(s2),
                _wrap16(xt), _wrap16(px),
            ],
            axis=1,
        )
        assert idx_pack.shape == (16, IDXW)
        in_maps.append(
            {
                "xs": xbf[c * SHARD : (c + 1) * SHARD, :],
                "idxall": idx_pack,
                "wr1T": np.ascontiguousarray(w_rel1.T).astype(bf16),
                "wo1T": np.ascontiguousarray(w_root1.T).astype(bf16),
                "wr2T": np.ascontiguousarray(w_rel2.T).astype(bf16),
                "wo2T": np.ascontiguousarray(w_root2.T).astype(bf16),
                "b1c": b1c,
                "b2r": b_rel2.reshape(1, O).astype(bf16),
                "ones": np.ones((1, 512), np.float32).astype(bf16),
                "identb": identb,
                "ident32": ident32,
            }
        )
    return in_maps


def get_nc():
    if "nc" not in _CACHED:
        _CACHED["nc"] = _build_program()
    return _CACHED["nc"]


def kernel(**inputs):
    from concourse.bass_utils import run_bass_kernel_spmd

    in_maps = prepare_in_maps(inputs)
    nc = get_nc()
    res = run_bass_kernel_spmd(nc, in_maps, core_ids=list(range(NC)), trace=False)
    out = np.concatenate(
        [res.results[c]["out"][:SHARD] for c in range(NC)], axis=0
    )
    return out.astype(np.float32)


# revision 15
# speedup vs baseline: 1.7030x; 1.5861x over previous
"""2-layer GraphConv GNN on 8 trn2 NeuronCores (Bass/Tile) — v4.

Design: aggregation entirely on the DMA stream (dma_gather + dma_scatter_add),
no per-edge compute instructions. ~700 instructions total.

  - Edges sharded by dst node (core c owns dst in [c*12500, (c+1)*12500)).
  - L1: gather x[src] rows (bf16, 256B) from the AllGather-built table,
    then dma_scatter_add them into agg1[dst] (bf16, DRAM). L2 gathers the
    hr table in f32 (256B rows; 64 cols in bf16 would be under the 256B
    elem minimum). Scatter calls are color-segmented (one occurrence index
    per (range,dst) per call) so all dst rows in a call are unique.
  - agg read back FEATURE-major in ONE transposed dma_gather (bf16, rows
    paired to satisfy the 256B elem minimum) -> dense 448/512-wide PE
    transforms, ACT relu+bias.
  - Internal node order sigma = [even nodes | odd nodes] so pair-stacked
    PSUM results transpose directly into natural node-major pair rows.
  - hr exchange: AllGather of per-core [12544, 64] f32 shards (padded to
    98*128); L2 gather indices account for the 12544 stride.
  - Upload: only bf16 x-shard + int16 idx pack (~5MB/core); output bf16.
"""

import numpy as np
import ml_dtypes
from contextlib import ExitStack

N = 100000
F = 128
O = 64
NC = 8
SHARD = N // NC          # 12500
SH2 = 12544              # padded shard rows (98*128) for hr/out
P2 = SH2 // 2            # 6272 sigma pair columns
PV = SHARD // 2          # 6250 valid pairs
PC = 6400                # padded pair count for transposed agg gathers
NR = 4
RS = N // NR             # 25000 (L1 gather ranges)
N2 = NC * SH2            # 100352 (hr_full rows)
RS2 = N2 // NR           # 25088 (L2 gather ranges)
CH = 5120                # rows per gather/scatter chunk

bf16 = ml_dtypes.bfloat16

# dynamic slot-layout profile, set by prepare_in_maps() before build:
# _PROF[L] = {"RB": range stride (mult of CH), "segs": [per-range list of
#             (start, len) color segments, 128-aligned]}
_PROF = {}


def _layout_consts():
    RB1, RB2 = _PROF[1]["RB"], _PROF[2]["RB"]
    TOT1, TOT2 = NR * RB1, NR * RB2
    OG1 = 0
    OS1 = OG1 + TOT1 // 16
    OG2 = OS1 + TOT1 // 16
    OS2 = OG2 + TOT2 // 16
    OXT = OS2 + TOT2 // 16
    OPX = OXT + SH2 // 16
    IDXW = OPX + PC // 16
    return RB1, RB2, TOT1, TOT2, OG1, OS1, OG2, OS2, OXT, OPX, IDXW

import os
_L1ONLY = bool(int(os.environ.get("GNN_L1ONLY", "0")))


def input_decls():
    IDXW = _layout_consts()[-1]
    return [
        ("xs", [SHARD, F], "bfloat16"),
        ("idxall", [16, IDXW], "int16"),
        ("wr1T", [F, F], "bfloat16"),
        ("wo1T", [F, F], "bfloat16"),
        ("wr2T", [F, O], "bfloat16"),
        ("wo2T", [F, O], "bfloat16"),
        ("b1c", [128, 1], "float32"),
        ("b2r", [1, O], "bfloat16"),
        ("ones", [1, 512], "bfloat16"),
        ("identb", [128, 128], "bfloat16"),
        ("ident32", [128, 128], "float32"),
    ]


def _build_program():
    import concourse.bass as bass
    import concourse.tile as tile
    from concourse import bacc, mybir

    RB1, RB2, TOT1, TOT2, OG1, OS1, OG2, OS2, OXT, OPX, IDXW = _layout_consts()
    nc = bacc.Bacc(None, target_bir_lowering=False, num_swdge_queues=4)
    dt = mybir.dt

    xs_in = nc.dram_tensor("xs", [SHARD, F], dt.bfloat16, kind="ExternalInput")
    idxall = nc.dram_tensor("idxall", [16, IDXW], dt.int16, kind="ExternalInput")
    wr1T = nc.dram_tensor("wr1T", [F, F], dt.bfloat16, kind="ExternalInput")
    wo1T = nc.dram_tensor("wo1T", [F, F], dt.bfloat16, kind="ExternalInput")
    wr2T = nc.dram_tensor("wr2T", [F, O], dt.bfloat16, kind="ExternalInput")
    wo2T = nc.dram_tensor("wo2T", [F, O], dt.bfloat16, kind="ExternalInput")
    b1c_in = nc.dram_tensor("b1c", [128, 1], dt.float32, kind="ExternalInput")
    b2r_in = nc.dram_tensor("b2r", [1, O], dt.bfloat16, kind="ExternalInput")
    ones_in = nc.dram_tensor("ones", [1, 512], dt.bfloat16, kind="ExternalInput")
    identb_in = nc.dram_tensor("identb", [128, 128], dt.bfloat16, kind="ExternalInput")
    ident32_in = nc.dram_tensor("ident32", [128, 128], dt.float32, kind="ExternalInput")
    out_t = nc.dram_tensor("out", [SH2, O], dt.bfloat16, kind="ExternalOutput")

    xs_int = nc.dram_tensor("xs_int", [SHARD, F], dt.bfloat16)
    xfull_bf = nc.dram_tensor("xfull_bf", [N, F], dt.bfloat16, addr_space="Shared")
    idxf = nc.dram_tensor("idxf", [128, IDXW], dt.int16)
    agg1 = nc.dram_tensor("agg1", [SHARD + 128, F], dt.bfloat16)
    hr_shard = nc.dram_tensor("hr_shard", [SH2, O], dt.float32)
    hr_full = nc.dram_tensor("hr_full", [N2, O], dt.float32, addr_space="Shared")
    agg2 = nc.dram_tensor("agg2", [SHARD + 128, O], dt.float32)
    agg2b = nc.dram_tensor("agg2b", [SHARD + 128, O], dt.bfloat16)

    with tile.TileContext(nc) as tc, ExitStack() as ctx:
        const_p = ctx.enter_context(tc.tile_pool(name="const", bufs=1))
        resid_p = ctx.enter_context(tc.tile_pool(name="resid", bufs=1))
        idx_p = ctx.enter_context(tc.tile_pool(name="idxp", bufs=2))
        msgs_p = ctx.enter_context(tc.tile_pool(name="msgs", bufs=2))
        sb_p = ctx.enter_context(tc.tile_pool(name="sbp", bufs=2))
        ps_h = ctx.enter_context(tc.tile_pool(name="ps_h", bufs=2, space="PSUM"))
        ps_stk = ctx.enter_context(tc.tile_pool(name="ps_stk", bufs=2, space="PSUM"))
        ps_tr = ctx.enter_context(tc.tile_pool(name="ps_tr", bufs=2, space="PSUM"))
        ps_trb = ctx.enter_context(tc.tile_pool(name="ps_trb", bufs=2, space="PSUM"))

        # ---- prologue ----
        nc.sync.dma_start(xs_int[:], xs_in[:])
        nc.gpsimd.collective_compute(
            "AllGather",
            mybir.AluOpType.bypass,
            replica_groups=[list(range(NC))],
            ins=[xs_int[:]],
            outs=[xfull_bf[:]],
        )
        # idx replication [16, W] -> [128, W]
        for k in range(8):
            nc.sync.dma_start(idxf[16 * k : 16 * (k + 1), :], idxall[:])

        c_wr1T = const_p.tile([F, F], dt.bfloat16)
        nc.sync.dma_start(c_wr1T[:], wr1T[:])
        c_wo1T = const_p.tile([F, F], dt.bfloat16)
        nc.sync.dma_start(c_wo1T[:], wo1T[:])
        c_wr2T = const_p.tile([F, O], dt.bfloat16)
        nc.sync.dma_start(c_wr2T[:], wr2T[:])
        c_wo2T = const_p.tile([F, O], dt.bfloat16)
        nc.sync.dma_start(c_wo2T[:], wo2T[:])
        c_b1c = const_p.tile([128, 1], dt.float32)
        nc.sync.dma_start(c_b1c[:], b1c_in[:])
        c_b2r = const_p.tile([1, O], dt.bfloat16)
        nc.sync.dma_start(c_b2r[:], b2r_in[:])
        c_ones = const_p.tile([1, 512], dt.bfloat16)
        nc.sync.dma_start(c_ones[:], ones_in[:])
        c_identb = const_p.tile([128, 128], dt.bfloat16)
        nc.sync.dma_start(c_identb[:], identb_in[:])
        c_ident32 = const_p.tile([128, 128], dt.float32)
        nc.sync.dma_start(c_ident32[:], ident32_in[:])

        # zero agg1 / agg2
        zt = const_p.tile([128, 2048], dt.float32)
        nc.vector.memset(zt[:], 0.0)
        ztb = const_p.tile([128, 2048], dt.bfloat16)
        nc.vector.memset(ztb[:], 0.0)
        a1_flat = agg1[:].rearrange("n f -> (n f)").rearrange("(a b) -> a b", a=128)
        W1 = a1_flat.shape[1]  # 12500
        for i in range(8):
            lo = i * 2048
            hi = min(W1, lo + 2048)
            if lo < W1:
                nc.sync.dma_start(a1_flat[:, lo:hi], ztb[:, : hi - lo])
        a2_flat = agg2[:].rearrange("n f -> (n f)").rearrange("(a b) -> a b", a=128)
        W2 = a2_flat.shape[1]  # 6250
        for i in range(4):
            lo = i * 2048
            hi = min(W2, lo + 2048)
            if lo < W2:
                nc.sync.dma_start(a2_flat[:, lo:hi], zt[:, : hi - lo])

        # r_xiT: sigma-ordered feature-major x shard via one transposed gather
        r_xiT = resid_p.tile([128, SH2], dt.bfloat16)
        xt_idx = idx_p.tile([128, SH2 // 16], dt.int16, tag="bigidx")
        nc.sync.dma_start(xt_idx[:], idxf[:, OXT : OXT + SH2 // 16])
        nc.gpsimd.dma_gather(
            r_xiT[:].rearrange("p (c e) -> p c e", c=1),
            xs_int[:],
            xt_idx[:],
            SH2,
            SH2,
            F,
            transpose=True,
            single_packet=False,
            queue_num=0,
        )
        r_hT = resid_p.tile([128, SH2], dt.bfloat16)
        aggT1 = resid_p.tile([128, 2 * PC], dt.bfloat16)
        aggT2 = resid_p.tile([128, PC], dt.bfloat16)
        px_idx = idx_p.tile([128, PC // 16], dt.int16, tag="pidx")
        nc.sync.dma_start(px_idx[:], idxf[:, OPX : OPX + PC // 16])

        # resident scatter idx for current layer
        sidx_res = resid_p.tile([128, max(TOT1, TOT2) // 16], dt.int16)

        def gs_chunks(L):
            """Gather fixed chunks; scatter_add per (color segment x chunk)
            intersection so every scatter call has unique dst rows
            (dma_scatter_add loses adds on duplicate idx within a call)."""
            OG = OG1 if L == 1 else OG2
            OS = OS1 if L == 1 else OS2
            FW = F if L == 1 else O
            mdt = dt.bfloat16 if L == 1 else dt.float32
            agg = agg1 if L == 1 else agg2
            table = xfull_bf if L == 1 else hr_full
            RSL = RS if L == 1 else RS2
            RB = RB1 if L == 1 else RB2
            segs = _PROF[L]["segs"]
            NCH = RB // CH
            CHC = CH // 16
            nc.sync.dma_start(
                sidx_res[:, : (NR * RB) // 16], idxf[:, OS : OS + (NR * RB) // 16]
            )
            mtiles = {}
            git = None
            for c in range(NR * NCH):
                r = c // NCH
                if c % NCH == 0:
                    git = idx_p.tile([128, RB // 16], dt.int16, tag="git")
                    nc.sync.dma_start(
                        git[:], idxf[:, OG + r * (RB // 16) : OG + (r + 1) * (RB // 16)]
                    )
                k = c % NCH
                m = msgs_p.tile([128, (CH // 128) * FW], mdt, tag="m")
                nc.gpsimd.dma_gather(
                    m[:].rearrange("p (c e) -> p c e", e=FW),
                    table[r * RSL : (r + 1) * RSL, :],
                    git[:, k * CHC : (k + 1) * CHC],
                    CH,
                    CH,
                    FW,
                    single_packet=False,
                    queue_num=0,
                )
                mtiles[c] = m
                # scatter every (segment x this-chunk) intersection
                clo, chi = c * CH, (c + 1) * CH
                base = r * RB
                for (sst, sln) in segs[r]:
                    a = max(base + sst, clo)
                    b = min(base + sst + sln, chi)
                    if a >= b:
                        continue
                    off = a - clo  # 128-aligned
                    nrow = b - a
                    nc.gpsimd.dma_scatter_add(
                        agg[:],
                        m[:].rearrange("p (c e) -> p c e", e=FW)[
                            :, off // 128 : off // 128 + nrow // 128, :
                        ],
                        sidx_res[:, a // 16 : a // 16 + nrow // 16],
                        nrow,
                        nrow,
                        FW,
                        single_packet=False,
                        queue_num=0,
                    )

        # ================= layer 1 =================
        gs_chunks(1)
        # aggT1: [128, 2, PC] via transposed gather of paired rows (512B)
        nc.gpsimd.dma_gather(
            aggT1[:].rearrange("p (c e) -> p c e", c=2),
            agg1[:].rearrange("(a b) f -> a (b f)", b=2),
            px_idx[:],
            PC,
            PC,
            2 * F,
            transpose=True,
            single_packet=False,
            queue_num=0,
        )
        # transform: h = relu(wr1@aggT + wo1@xT + b1), 28 batches of 448
        aggT1v = aggT1[:].rearrange("p (c e) -> p c e", c=2)
        for b in range(28):
            plane = b // 14
            lo = (b % 14) * 448
            ph = ps_h.tile([128, 512], dt.float32, tag="ph", space="PSUM")
            nc.tensor.matmul(
                ph[:, :448],
                lhsT=c_wr1T[:],
                rhs=aggT1v[:, plane, lo : lo + 448],
                start=True,
                stop=False,
            )
            nc.tensor.matmul(
                ph[:, :448],
                lhsT=c_wo1T[:],
                rhs=r_xiT[:, plane * P2 + lo : plane * P2 + lo + 448],
                start=False,
                stop=True,
            )
            nc.scalar.activation(
                out=r_hT[:, plane * P2 + lo : plane * P2 + lo + 448],
                in_=ph[:, :448],
                func=mybir.ActivationFunctionType.Relu,
                bias=c_b1c[:],
            )
        # hr = wr2 @ h, pair-stacked -> transpose -> node-major pair rows
        hr_pairs = hr_shard[:].rearrange("(q t) o -> q (t o)", t=2)  # [6272, 128]
        for b in range(13):
            lo = b * 512
            w = 512 if b < 12 else P2 - 12 * 512  # 128
            pstk = ps_stk.tile([128, 512], dt.float32, tag="stk", space="PSUM")
            nc.tensor.matmul(
                pstk[0:64, :w], lhsT=c_wr2T[:], rhs=r_hT[:, lo : lo + w],
                start=True, stop=True,
            )
            nc.tensor.matmul(
                pstk[64:128, :w], lhsT=c_wr2T[:], rhs=r_hT[:, P2 + lo : P2 + lo + w],
                start=True, stop=True,
            )
            stk_sb = sb_p.tile([128, 512], dt.float32, tag="stks")
            nc.scalar.copy(out=stk_sb[:, :w], in_=pstk[:, :w])
            ptr = ps_tr.tile([128, 512], dt.float32, tag="tr", space="PSUM")
            for t in range(w // 128):
                nc.tensor.transpose(
                    ptr[:, t * 128 : (t + 1) * 128],
                    stk_sb[:, t * 128 : (t + 1) * 128],
                    c_ident32[:],
                )
            hw_sb = sb_p.tile([128, 512], dt.float32, tag="hws")
            nc.scalar.copy(out=hw_sb[:, :w], in_=ptr[:, :w])
            nc.sync.dma_start(
                hr_pairs[lo : lo + w, :].rearrange("(c p) e -> p c e", p=128),
                hw_sb[:, :w].rearrange("p (c e) -> p c e", e=128),
            )

        if _L1ONLY:
            zo = sb_p.tile([128, O], dt.bfloat16, tag="zo")
            nc.vector.memset(zo[:], 0.0)
            nc.sync.dma_start(out_t[0:128, :], zo[:])
        else:
            # ================= exchange =================
            nc.gpsimd.collective_compute(
                "AllGather",
                mybir.AluOpType.bypass,
                replica_groups=[list(range(NC))],
                ins=[hr_shard[:]],
                outs=[hr_full[:]],
            )
            # ================= layer 2 =================
            gs_chunks(2)
            a2b_flat = agg2b[:].rearrange("n f -> (n f)").rearrange("(a b) -> a b", a=128)
            nc.gpsimd.dma_start(a2b_flat[:], a2_flat[:])
            # aggT2: stacked [128, PC] (paired 256B rows)
            nc.gpsimd.dma_gather(
                aggT2[:].rearrange("p (c e) -> p c e", c=1),
                agg2b[:].rearrange("(a b) f -> a (b f)", b=2),
                px_idx[:],
                PC,
                PC,
                2 * O,
                transpose=True,
                single_packet=False,
                queue_num=0,
            )
            # out = agg2 + wo2@h + b2, pair-stacked
            out_pairs = out_t[:].rearrange("(q t) o -> q (t o)", t=2)  # [6272, 128]
            for b in range(13):
                lo = b * 512
                w = 512 if b < 12 else P2 - 12 * 512
                pstk = ps_stk.tile([128, 512], dt.float32, tag="stk", space="PSUM")
                nc.tensor.matmul(
                    pstk[0:64, :w], lhsT=c_wo2T[:], rhs=r_hT[:, lo : lo + w],
                    start=True, stop=False,
                )
                nc.tensor.matmul(
                    pstk[0:64, :w], lhsT=c_b2r[:1, :], rhs=c_ones[:1, :w],
                    start=False, stop=True,
                )
                nc.tensor.matmul(
                    pstk[64:128, :w], lhsT=c_wo2T[:], rhs=r_hT[:, P2 + lo : P2 + lo + w],
                    start=True, stop=False,
                )
                nc.tensor.matmul(
                    pstk[64:128, :w], lhsT=c_b2r[:1, :], rhs=c_ones[:1, :w],
                    start=False, stop=True,
                )
                stk_sb = sb_p.tile([128, 512], dt.bfloat16, tag="stk2s")
                nc.vector.tensor_tensor(
                    out=stk_sb[:, :w],
                    in0=pstk[:, :w],
                    in1=aggT2[:, lo : lo + w],
                    op=mybir.AluOpType.add,
                )
                ptrb = ps_trb.tile([128, 512], dt.bfloat16, tag="trb", space="PSUM")
                for t in range(w // 128):
                    nc.tensor.transpose(
                        ptrb[:, t * 128 : (t + 1) * 128],
                        stk_sb[:, t * 128 : (t + 1) * 128],
                        c_identb[:],
                    )
                ob_sb = sb_p.tile([128, 512], dt.bfloat16, tag="obs")
                nc.scalar.copy(out=ob_sb[:, :w], in_=ptrb[:, :w])
                nc.sync.dma_start(
                    out_pairs[lo : lo + w, :].rearrange("(c p) e -> p c e", p=128),
                    ob_sb[:, :w].rearrange("p (c e) -> p c e", e=128),
                )

    nc.finalize()
    return nc


_CACHED = {}


def _wrap16(flat):
    """[K] int16 -> [16, K//16], slot i at (i%16, i//16)."""
    return np.ascontiguousarray(flat.reshape(-1, 16).T)


def _rup(x, m):
    return (x + m - 1) // m * m


def _occ_split(gv, d, rr):
    """(r, dst)-stable order + per-(r,dst) occurrence index, one global sort."""
    keyA = (rr * SHARD + d).astype(np.int32)
    oA = np.argsort(keyA, kind="stable")
    g_o, d_o, r_o = gv[oA], d[oA], rr[oA]
    grp = keyA[oA]
    cnt = np.bincount(grp, minlength=NR * SHARD)
    st = np.zeros(NR * SHARD + 1, dtype=np.int64)
    np.cumsum(cnt, out=st[1:])
    occ = np.arange(len(grp)) - st[grp]
    return g_o, d_o, r_o, occ


def _build_profile(all_splits):
    """all_splits: per core (g_o, d_o, r_o, occ). Returns {"RB", "segs"} with
    128-aligned color-segment budgets = max over cores + margin."""
    maxk = max(
        (int(occ.max()) + 1 if len(occ) else 0) for (_, _, _, occ) in all_splits
    )
    sizes = np.zeros((NR, maxk), dtype=np.int64)
    for (_, _, r_o, occ) in all_splits:
        c = np.bincount(r_o * maxk + occ, minlength=NR * maxk).reshape(NR, maxk)
        sizes = np.maximum(sizes, c)
    budgets = (sizes + 128 + 127) // 128 * 128
    segs = []
    for r in range(NR):
        st = np.zeros(maxk + 1, dtype=np.int64)
        np.cumsum(budgets[r], out=st[1:])
        segs.append([(int(st[k]), int(budgets[r][k])) for k in range(maxk)])
    RB = _rup(int(max(np.sum(budgets[r]) for r in range(NR))), CH)
    return {"RB": RB, "segs": segs}


def _fill_layout(split, prof):
    """Returns (g [TOT], s [TOT]) int16; gather pad idx 0, scatter pad dump."""
    RB = prof["RB"]
    segs = prof["segs"]
    MK = len(segs[0])
    TOT = NR * RB
    g_o, d_o, r_o, occ = split
    keyB = (r_o * MK + occ).astype(np.int32)
    oB = np.argsort(keyB, kind="stable")
    kB = keyB[oB]
    cnt = np.bincount(kB, minlength=NR * MK)
    st = np.zeros(NR * MK + 1, dtype=np.int64)
    np.cumsum(cnt, out=st[1:])
    pos = np.arange(len(kB)) - st[kB]
    segstart = np.array(
        [segs[r][k][0] for r in range(NR) for k in range(MK)], dtype=np.int64
    )
    seglen = np.array(
        [segs[r][k][1] for r in range(NR) for k in range(MK)], dtype=np.int64
    )
    if np.any(cnt > seglen):
        raise RuntimeError("color segment overflow")
    slot = r_o[oB] * RB + segstart[kB] + pos
    g = np.zeros(TOT, dtype=np.int16)
    s = np.full(TOT, SHARD, dtype=np.int16)  # pad -> dump row (races harmless)
    g[slot] = g_o[oB].astype(np.int16)
    s[slot] = d_o[oB].astype(np.int16)
    return g, s


def prepare_in_maps(inputs):
    x = np.asarray(inputs["x"], dtype=np.float32)
    edge_index = np.asarray(inputs["edge_index"])
    w_rel1 = np.asarray(inputs["w_rel1"], dtype=np.float32)
    b_rel1 = np.asarray(inputs["b_rel1"], dtype=np.float32)
    w_root1 = np.asarray(inputs["w_root1"], dtype=np.float32)
    w_rel2 = np.asarray(inputs["w_rel2"], dtype=np.float32)
    b_rel2 = np.asarray(inputs["b_rel2"], dtype=np.float32)
    w_root2 = np.asarray(inputs["w_root2"], dtype=np.float32)

    src = edge_index[0].astype(np.int32)
    dst = edge_index[1].astype(np.int32)
    xbf = x.astype(bf16)

    # sigma-ordered xiT gather idx + pair idx (same for all cores except xiT)
    xt = np.zeros(SH2, dtype=np.int16)
    xt[0:PV] = (np.arange(PV) * 2).astype(np.int16)
    xt[P2 : P2 + PV] = (np.arange(PV) * 2 + 1).astype(np.int16)
    px = np.zeros(PC, dtype=np.int16)
    px[0:PV] = np.arange(PV, dtype=np.int16)

    identb = np.eye(128, dtype=np.float32).astype(bf16)
    ident32 = np.eye(128, dtype=np.float32)
    b1c = np.zeros((128, 1), np.float32)
    b1c[: len(b_rel1), 0] = b_rel1

    core = (dst // SHARD).astype(np.int32)
    order = np.argsort(core, kind="stable")
    src_s, dst_s = src[order], dst[order]
    bounds = np.searchsorted(core[order], np.arange(NC + 1))

    # pass 1: per-core per-range occurrence splits for both layers
    splits1, splits2 = [], []
    for c in range(NC):
        lo, hi = bounds[c], bounds[c + 1]
        sc, dc = src_s[lo:hi], dst_s[lo:hi] - c * SHARD
        splits1.append(_occ_split(sc % RS, dc, sc // RS))
        gsrc2 = (sc // SHARD) * SH2 + (sc % SHARD)
        splits2.append(_occ_split(gsrc2 % RS2, dc, gsrc2 // RS2))
    _PROF[1] = _build_profile(splits1)
    _PROF[2] = _build_profile(splits2)
    IDXW = _layout_consts()[-1]

    in_maps = []
    for c in range(NC):
        g1, s1 = _fill_layout(splits1[c], _PROF[1])
        g2, s2 = _fill_layout(splits2[c], _PROF[2])
        idx_pack = np.concatenate(
            [
                _wrap16(g1), _wrap16(s1), _wrap16(g2), _wrap16(s2),
                _wrap16(xt), _wrap16(px),
            ],
            axis=1,
        )
        assert idx_pack.shape == (16, IDXW)
        in_maps.append(
            {
                "xs": xbf[c * SHARD : (c + 1) * SHARD, :],
                "idxall": idx_pack,
                "wr1T": np.ascontiguousarray(w_rel1.T).astype(bf16),
                "wo1T": np.ascontiguousarray(w_root1.T).astype(bf16),
                "wr2T": np.ascontiguousarray(w_rel2.T).astype(bf16),
                "wo2T": np.ascontiguousarray(w_root2.T).astype(bf16),
                "b1c": b1c,
                "b2r": b_rel2.reshape(1, O).astype(bf16),
                "ones": np.ones((1, 512), np.float32).astype(bf16),
                "identb": identb,
                "ident32": ident32,
            }
        )
    return in_maps


def get_nc():
    if "nc" not in _CACHED:
        _CACHED["nc"] = _build_program()
    return _CACHED["nc"]


def kernel(**inputs):
    from concourse.bass_utils import run_bass_kernel_spmd

    in_maps = prepare_in_maps(inputs)
    nc = get_nc()
    res = run_bass_kernel_spmd(nc, in_maps, core_ids=list(range(NC)), trace=False)
    out = np.concatenate(
        [res.results[c]["out"][:SHARD] for c in range(NC)], axis=0
    )
    return out.astype(np.float32)


# revision 16
# speedup vs baseline: 1.7455x; 1.0250x over previous
"""2-layer GraphConv GNN on 8 trn2 NeuronCores (Bass/Tile) — v4.

Design: aggregation entirely on the DMA stream (dma_gather + dma_scatter_add),
no per-edge compute instructions. ~700 instructions total.

  - Edges sharded by dst node (core c owns dst in [c*12500, (c+1)*12500)).
  - L1: gather x[src] rows (bf16, 256B) from the AllGather-built table,
    then dma_scatter_add them into agg1[dst] (bf16, DRAM). L2 gathers the
    hr table in f32 (256B rows; 64 cols in bf16 would be under the 256B
    elem minimum). Scatter calls are color-segmented (one occurrence index
    per (range,dst) per call) so all dst rows in a call are unique.
  - agg read back FEATURE-major in ONE transposed dma_gather (bf16, rows
    paired to satisfy the 256B elem minimum) -> dense 448/512-wide PE
    transforms, ACT relu+bias.
  - Internal node order sigma = [even nodes | odd nodes] so pair-stacked
    PSUM results transpose directly into natural node-major pair rows.
  - hr exchange: AllGather of per-core [12544, 64] f32 shards (padded to
    98*128); L2 gather indices account for the 12544 stride.
  - Upload: only bf16 x-shard + int16 idx pack (~5MB/core); output bf16.
"""

import numpy as np
import ml_dtypes
from contextlib import ExitStack

N = 100000
F = 128
O = 64
NC = 8
SHARD = N // NC          # 12500
SH2 = 12544              # padded shard rows (98*128) for hr/out
P2 = SH2 // 2            # 6272 sigma pair columns
PV = SHARD // 2          # 6250 valid pairs
PC = 6400                # padded pair count for transposed agg gathers
NR = 4
RS = N // NR             # 25000 (L1 gather ranges)
N2 = NC * SH2            # 100352 (hr_full rows)
RS2 = N2 // NR           # 25088 (L2 gather ranges)
CH = 5120                # rows per gather/scatter chunk

bf16 = ml_dtypes.bfloat16

# dynamic slot-layout profile, set by prepare_in_maps() before build:
# _PROF[L] = {"RB": range stride (mult of CH), "segs": [per-range list of
#             (start, len) color segments, 128-aligned]}
_PROF = {}


def _layout_consts():
    RB1, RB2 = _PROF[1]["RB"], _PROF[2]["RB"]
    TOT1, TOT2 = NR * RB1, NR * RB2
    OG1 = 0
    OS1 = OG1 + TOT1 // 16
    OG2 = OS1 + TOT1 // 16
    OS2 = OG2 + TOT2 // 16
    OXT = OS2 + TOT2 // 16
    OPX = OXT + SH2 // 16
    IDXW = OPX + PC // 16
    return RB1, RB2, TOT1, TOT2, OG1, OS1, OG2, OS2, OXT, OPX, IDXW

import os
_L1ONLY = bool(int(os.environ.get("GNN_L1ONLY", "0")))


def input_decls():
    IDXW = _layout_consts()[-1]
    return [
        ("xs", [SHARD, F], "bfloat16"),
        ("idxall", [16, IDXW], "int16"),
        ("wr1T", [F, F], "bfloat16"),
        ("wo1T", [F, F], "bfloat16"),
        ("wr2T", [F, O], "bfloat16"),
        ("wo2T", [F, O], "bfloat16"),
        ("b1c", [128, 1], "float32"),
        ("b2r", [1, O], "bfloat16"),
        ("ones", [1, 512], "bfloat16"),
        ("identb", [128, 128], "bfloat16"),
        ("ident32", [128, 128], "float32"),
    ]


def _build_program():
    import concourse.bass as bass
    import concourse.tile as tile
    from concourse import bacc, mybir

    RB1, RB2, TOT1, TOT2, OG1, OS1, OG2, OS2, OXT, OPX, IDXW = _layout_consts()
    nc = bacc.Bacc(None, target_bir_lowering=False, num_swdge_queues=4)
    dt = mybir.dt

    xs_in = nc.dram_tensor("xs", [SHARD, F], dt.bfloat16, kind="ExternalInput")
    idxall = nc.dram_tensor("idxall", [16, IDXW], dt.int16, kind="ExternalInput")
    wr1T = nc.dram_tensor("wr1T", [F, F], dt.bfloat16, kind="ExternalInput")
    wo1T = nc.dram_tensor("wo1T", [F, F], dt.bfloat16, kind="ExternalInput")
    wr2T = nc.dram_tensor("wr2T", [F, O], dt.bfloat16, kind="ExternalInput")
    wo2T = nc.dram_tensor("wo2T", [F, O], dt.bfloat16, kind="ExternalInput")
    b1c_in = nc.dram_tensor("b1c", [128, 1], dt.float32, kind="ExternalInput")
    b2r_in = nc.dram_tensor("b2r", [1, O], dt.bfloat16, kind="ExternalInput")
    ones_in = nc.dram_tensor("ones", [1, 512], dt.bfloat16, kind="ExternalInput")
    identb_in = nc.dram_tensor("identb", [128, 128], dt.bfloat16, kind="ExternalInput")
    ident32_in = nc.dram_tensor("ident32", [128, 128], dt.float32, kind="ExternalInput")
    out_t = nc.dram_tensor("out", [SH2, O], dt.bfloat16, kind="ExternalOutput")

    xs_int = nc.dram_tensor("xs_int", [SHARD, F], dt.bfloat16)
    xfull_bf = nc.dram_tensor("xfull_bf", [N, F], dt.bfloat16, addr_space="Shared")
    idxf = nc.dram_tensor("idxf", [128, IDXW], dt.int16)
    agg1 = nc.dram_tensor("agg1", [SHARD + 128, F], dt.bfloat16)
    hr_shard = nc.dram_tensor("hr_shard", [SH2, O], dt.float32)
    hr_full = nc.dram_tensor("hr_full", [N2, O], dt.float32, addr_space="Shared")
    agg2 = nc.dram_tensor("agg2", [SHARD + 128, O], dt.float32)
    agg2b = nc.dram_tensor("agg2b", [SHARD + 128, O], dt.bfloat16)

    with tile.TileContext(nc) as tc, ExitStack() as ctx:
        const_p = ctx.enter_context(tc.tile_pool(name="const", bufs=1))
        resid_p = ctx.enter_context(tc.tile_pool(name="resid", bufs=1))
        idx_p = ctx.enter_context(tc.tile_pool(name="idxp", bufs=2))
        msgs_p = ctx.enter_context(tc.tile_pool(name="msgs", bufs=2))
        sb_p = ctx.enter_context(tc.tile_pool(name="sbp", bufs=2))
        ps_h = ctx.enter_context(tc.tile_pool(name="ps_h", bufs=2, space="PSUM"))
        ps_stk = ctx.enter_context(tc.tile_pool(name="ps_stk", bufs=2, space="PSUM"))
        ps_tr = ctx.enter_context(tc.tile_pool(name="ps_tr", bufs=2, space="PSUM"))
        ps_trb = ctx.enter_context(tc.tile_pool(name="ps_trb", bufs=2, space="PSUM"))

        # ---- prologue ----
        nc.sync.dma_start(xs_int[:], xs_in[:])
        nc.gpsimd.collective_compute(
            "AllGather",
            mybir.AluOpType.bypass,
            replica_groups=[list(range(NC))],
            ins=[xs_int[:]],
            outs=[xfull_bf[:]],
        )
        # idx replication [16, W] -> [128, W]
        for k in range(8):
            nc.sync.dma_start(idxf[16 * k : 16 * (k + 1), :], idxall[:])

        c_wr1T = const_p.tile([F, F], dt.bfloat16)
        nc.sync.dma_start(c_wr1T[:], wr1T[:])
        c_wo1T = const_p.tile([F, F], dt.bfloat16)
        nc.sync.dma_start(c_wo1T[:], wo1T[:])
        c_wr2T = const_p.tile([F, O], dt.bfloat16)
        nc.sync.dma_start(c_wr2T[:], wr2T[:])
        c_wo2T = const_p.tile([F, O], dt.bfloat16)
        nc.sync.dma_start(c_wo2T[:], wo2T[:])
        c_b1c = const_p.tile([128, 1], dt.float32)
        nc.sync.dma_start(c_b1c[:], b1c_in[:])
        c_b2r = const_p.tile([1, O], dt.bfloat16)
        nc.sync.dma_start(c_b2r[:], b2r_in[:])
        c_ones = const_p.tile([1, 512], dt.bfloat16)
        nc.sync.dma_start(c_ones[:], ones_in[:])
        c_identb = const_p.tile([128, 128], dt.bfloat16)
        nc.sync.dma_start(c_identb[:], identb_in[:])
        c_ident32 = const_p.tile([128, 128], dt.float32)
        nc.sync.dma_start(c_ident32[:], ident32_in[:])

        # zero agg1 / agg2
        zt = const_p.tile([128, 2048], dt.float32)
        nc.vector.memset(zt[:], 0.0)
        ztb = const_p.tile([128, 2048], dt.bfloat16)
        nc.vector.memset(ztb[:], 0.0)
        a1_flat = agg1[:].rearrange("n f -> (n f)").rearrange("(a b) -> a b", a=128)
        W1 = a1_flat.shape[1]  # 12500
        for i in range(8):
            lo = i * 2048
            hi = min(W1, lo + 2048)
            if lo < W1:
                nc.sync.dma_start(a1_flat[:, lo:hi], ztb[:, : hi - lo])
        a2_flat = agg2[:].rearrange("n f -> (n f)").rearrange("(a b) -> a b", a=128)
        W2 = a2_flat.shape[1]  # 6250
        for i in range(4):
            lo = i * 2048
            hi = min(W2, lo + 2048)
            if lo < W2:
                nc.sync.dma_start(a2_flat[:, lo:hi], zt[:, : hi - lo])

        # r_xiT: sigma-ordered feature-major x shard via one transposed gather
        r_xiT = resid_p.tile([128, SH2], dt.bfloat16)
        xt_idx = idx_p.tile([128, SH2 // 16], dt.int16, tag="bigidx")
        nc.sync.dma_start(xt_idx[:], idxf[:, OXT : OXT + SH2 // 16])
        nc.gpsimd.dma_gather(
            r_xiT[:].rearrange("p (c e) -> p c e", c=1),
            xs_int[:],
            xt_idx[:],
            SH2,
            SH2,
            F,
            transpose=True,
            single_packet=False,
            queue_num=0,
        )
        r_hT = resid_p.tile([128, SH2], dt.bfloat16)
        aggT1 = resid_p.tile([128, 2 * PC], dt.bfloat16)
        aggT2 = resid_p.tile([128, PC], dt.bfloat16)
        px_idx = idx_p.tile([128, PC // 16], dt.int16, tag="pidx")
        nc.sync.dma_start(px_idx[:], idxf[:, OPX : OPX + PC // 16])

        # resident scatter idx for current layer
        sidx_res = resid_p.tile([128, max(TOT1, TOT2) // 16], dt.int16)

        def gs_chunks(L):
            """Gather fixed chunks; scatter_add per (color segment x chunk)
            intersection so every scatter call has unique dst rows
            (dma_scatter_add loses adds on duplicate idx within a call)."""
            OG = OG1 if L == 1 else OG2
            OS = OS1 if L == 1 else OS2
            FW = F if L == 1 else O
            mdt = dt.bfloat16 if L == 1 else dt.float32
            agg = agg1 if L == 1 else agg2
            table = xfull_bf if L == 1 else hr_full
            RSL = RS if L == 1 else RS2
            RB = RB1 if L == 1 else RB2
            segs = _PROF[L]["segs"]
            NCH = RB // CH
            CHC = CH // 16
            nc.sync.dma_start(
                sidx_res[:, : (NR * RB) // 16], idxf[:, OS : OS + (NR * RB) // 16]
            )
            for r in range(NR):
                used = segs[r][-1][0] + segs[r][-1][1]  # sum of budgets
                used = min(RB, (used + 127) // 128 * 128)
                nch = (used + CH - 1) // CH
                git = idx_p.tile([128, RB // 16], dt.int16, tag="git")
                nc.sync.dma_start(
                    git[:], idxf[:, OG + r * (RB // 16) : OG + (r + 1) * (RB // 16)]
                )
                for k in range(nch):
                    cbase = k * CH
                    nrow_g = min(CH, used - cbase)  # mult of 128
                    m = msgs_p.tile([128, (CH // 128) * FW], mdt, tag="m")
                    nc.gpsimd.dma_gather(
                        m[:].rearrange("p (c e) -> p c e", e=FW)[
                            :, : nrow_g // 128, :
                        ],
                        table[r * RSL : (r + 1) * RSL, :],
                        git[:, k * CHC : k * CHC + nrow_g // 16],
                        nrow_g,
                        nrow_g,
                        FW,
                        single_packet=False,
                        queue_num=0,
                    )
                    # scatter every (segment x this-chunk) intersection
                    clo, chi = cbase, cbase + nrow_g
                    base = r * RB
                    for (sst, sln) in segs[r]:
                        a = max(sst, clo)
                        b = min(sst + sln, chi)
                        if a >= b:
                            continue
                        off = a - clo  # 128-aligned
                        nrow = b - a
                        nc.gpsimd.dma_scatter_add(
                            agg[:],
                            m[:].rearrange("p (c e) -> p c e", e=FW)[
                                :, off // 128 : off // 128 + nrow // 128, :
                            ],
                            sidx_res[:, (base + a) // 16 : (base + a) // 16 + nrow // 16],
                            nrow,
                            nrow,
                            FW,
                            single_packet=False,
                            queue_num=0,
                        )

        # ================= layer 1 =================
        gs_chunks(1)
        # aggT1: [128, 2, PC] via transposed gather of paired rows (512B)
        nc.gpsimd.dma_gather(
            aggT1[:].rearrange("p (c e) -> p c e", c=2),
            agg1[:].rearrange("(a b) f -> a (b f)", b=2),
            px_idx[:],
            PC,
            PC,
            2 * F,
            transpose=True,
            single_packet=False,
            queue_num=0,
        )
        # transform: h = relu(wr1@aggT + wo1@xT + b1), 28 batches of 448
        aggT1v = aggT1[:].rearrange("p (c e) -> p c e", c=2)
        for b in range(28):
            plane = b // 14
            lo = (b % 14) * 448
            ph = ps_h.tile([128, 512], dt.float32, tag="ph", space="PSUM")
            nc.tensor.matmul(
                ph[:, :448],
                lhsT=c_wr1T[:],
                rhs=aggT1v[:, plane, lo : lo + 448],
                start=True,
                stop=False,
            )
            nc.tensor.matmul(
                ph[:, :448],
                lhsT=c_wo1T[:],
                rhs=r_xiT[:, plane * P2 + lo : plane * P2 + lo + 448],
                start=False,
                stop=True,
            )
            nc.scalar.activation(
                out=r_hT[:, plane * P2 + lo : plane * P2 + lo + 448],
                in_=ph[:, :448],
                func=mybir.ActivationFunctionType.Relu,
                bias=c_b1c[:],
            )
        # hr = wr2 @ h, pair-stacked -> transpose -> node-major pair rows
        hr_pairs = hr_shard[:].rearrange("(q t) o -> q (t o)", t=2)  # [6272, 128]
        for b in range(13):
            lo = b * 512
            w = 512 if b < 12 else P2 - 12 * 512  # 128
            pstk = ps_stk.tile([128, 512], dt.float32, tag="stk", space="PSUM")
            nc.tensor.matmul(
                pstk[0:64, :w], lhsT=c_wr2T[:], rhs=r_hT[:, lo : lo + w],
                start=True, stop=True,
            )
            nc.tensor.matmul(
                pstk[64:128, :w], lhsT=c_wr2T[:], rhs=r_hT[:, P2 + lo : P2 + lo + w],
                start=True, stop=True,
            )
            stk_sb = sb_p.tile([128, 512], dt.float32, tag="stks")
            nc.scalar.copy(out=stk_sb[:, :w], in_=pstk[:, :w])
            ptr = ps_tr.tile([128, 512], dt.float32, tag="tr", space="PSUM")
            for t in range(w // 128):
                nc.tensor.transpose(
                    ptr[:, t * 128 : (t + 1) * 128],
                    stk_sb[:, t * 128 : (t + 1) * 128],
                    c_ident32[:],
                )
            hw_sb = sb_p.tile([128, 512], dt.float32, tag="hws")
            nc.scalar.copy(out=hw_sb[:, :w], in_=ptr[:, :w])
            nc.sync.dma_start(
                hr_pairs[lo : lo + w, :].rearrange("(c p) e -> p c e", p=128),
                hw_sb[:, :w].rearrange("p (c e) -> p c e", e=128),
            )

        if _L1ONLY:
            zo = sb_p.tile([128, O], dt.bfloat16, tag="zo")
            nc.vector.memset(zo[:], 0.0)
            nc.sync.dma_start(out_t[0:128, :], zo[:])
        else:
            # ================= exchange =================
            nc.gpsimd.collective_compute(
                "AllGather",
                mybir.AluOpType.bypass,
                replica_groups=[list(range(NC))],
                ins=[hr_shard[:]],
                outs=[hr_full[:]],
            )
            # ================= layer 2 =================
            gs_chunks(2)
            a2b_flat = agg2b[:].rearrange("n f -> (n f)").rearrange("(a b) -> a b", a=128)
            nc.gpsimd.dma_start(a2b_flat[:], a2_flat[:])
            # aggT2: stacked [128, PC] (paired 256B rows)
            nc.gpsimd.dma_gather(
                aggT2[:].rearrange("p (c e) -> p c e", c=1),
                agg2b[:].rearrange("(a b) f -> a (b f)", b=2),
                px_idx[:],
                PC,
                PC,
                2 * O,
                transpose=True,
                single_packet=False,
                queue_num=0,
            )
            # out = agg2 + wo2@h + b2, pair-stacked
            out_pairs = out_t[:].rearrange("(q t) o -> q (t o)", t=2)  # [6272, 128]
            for b in range(13):
                lo = b * 512
                w = 512 if b < 12 else P2 - 12 * 512
                pstk = ps_stk.tile([128, 512], dt.float32, tag="stk", space="PSUM")
                nc.tensor.matmul(
                    pstk[0:64, :w], lhsT=c_wo2T[:], rhs=r_hT[:, lo : lo + w],
                    start=True, stop=False,
                )
                nc.tensor.matmul(
                    pstk[0:64, :w], lhsT=c_b2r[:1, :], rhs=c_ones[:1, :w],
                    start=False, stop=True,
                )
                nc.tensor.matmul(
                    pstk[64:128, :w], lhsT=c_wo2T[:], rhs=r_hT[:, P2 + lo : P2 + lo + w],
                    start=True, stop=False,
                )
                nc.tensor.matmul(
                    pstk[64:128, :w], lhsT=c_b2r[:1, :], rhs=c_ones[:1, :w],
                    start=False, stop=True,
                )
                stk_sb = sb_p.tile([128, 512], dt.bfloat16, tag="stk2s")
                nc.vector.tensor_tensor(
                    out=stk_sb[:, :w],
                    in0=pstk[:, :w],
                    in1=aggT2[:, lo : lo + w],
                    op=mybir.AluOpType.add,
                )
                ptrb = ps_trb.tile([128, 512], dt.bfloat16, tag="trb", space="PSUM")
                for t in range(w // 128):
                    nc.tensor.transpose(
                        ptrb[:, t * 128 : (t + 1) * 128],
                        stk_sb[:, t * 128 : (t + 1) * 128],
                        c_identb[:],
                    )
                ob_sb = sb_p.tile([128, 512], dt.bfloat16, tag="obs")
                nc.scalar.copy(out=ob_sb[:, :w], in_=ptrb[:, :w])
                nc.sync.dma_start(
                    out_pairs[lo : lo + w, :].rearrange("(c p) e -> p c e", p=128),
                    ob_sb[:, :w].rearrange("p (c e) -> p c e", e=128),
                )

    nc.finalize()
    return nc


_CACHED = {}


def _wrap16(flat):
    """[K] int16 -> [16, K//16], slot i at (i%16, i//16)."""
    return np.ascontiguousarray(flat.reshape(-1, 16).T)


def _rup(x, m):
    return (x + m - 1) // m * m


def _occ_split(gv, d, rr):
    """(r, dst)-stable order + per-(r,dst) occurrence index, one global sort."""
    keyA = (rr * SHARD + d).astype(np.int32)
    oA = np.argsort(keyA, kind="stable")
    g_o, d_o, r_o = gv[oA], d[oA], rr[oA]
    grp = keyA[oA]
    cnt = np.bincount(grp, minlength=NR * SHARD)
    st = np.zeros(NR * SHARD + 1, dtype=np.int64)
    np.cumsum(cnt, out=st[1:])
    occ = np.arange(len(grp)) - st[grp]
    return g_o, d_o, r_o, occ


def _build_profile(all_splits):
    """all_splits: per core (g_o, d_o, r_o, occ). Returns {"RB", "segs"} with
    128-aligned color-segment budgets = max over cores + margin."""
    maxk = max(
        (int(occ.max()) + 1 if len(occ) else 0) for (_, _, _, occ) in all_splits
    )
    sizes = np.zeros((NR, maxk), dtype=np.int64)
    for (_, _, r_o, occ) in all_splits:
        c = np.bincount(r_o * maxk + occ, minlength=NR * maxk).reshape(NR, maxk)
        sizes = np.maximum(sizes, c)
    budgets = (sizes + 127) // 128 * 128
    segs = []
    for r in range(NR):
        st = np.zeros(maxk + 1, dtype=np.int64)
        np.cumsum(budgets[r], out=st[1:])
        segs.append([(int(st[k]), int(budgets[r][k])) for k in range(maxk)])
    RB = _rup(int(max(np.sum(budgets[r]) for r in range(NR))), CH)
    return {"RB": RB, "segs": segs}


def _fill_layout(split, prof):
    """Returns (g [TOT], s [TOT]) int16; gather pad idx 0, scatter pad dump."""
    RB = prof["RB"]
    segs = prof["segs"]
    MK = len(segs[0])
    TOT = NR * RB
    g_o, d_o, r_o, occ = split
    keyB = (r_o * MK + occ).astype(np.int32)
    oB = np.argsort(keyB, kind="stable")
    kB = keyB[oB]
    cnt = np.bincount(kB, minlength=NR * MK)
    st = np.zeros(NR * MK + 1, dtype=np.int64)
    np.cumsum(cnt, out=st[1:])
    pos = np.arange(len(kB)) - st[kB]
    segstart = np.array(
        [segs[r][k][0] for r in range(NR) for k in range(MK)], dtype=np.int64
    )
    seglen = np.array(
        [segs[r][k][1] for r in range(NR) for k in range(MK)], dtype=np.int64
    )
    if np.any(cnt > seglen):
        raise RuntimeError("color segment overflow")
    slot = r_o[oB] * RB + segstart[kB] + pos
    g = np.zeros(TOT, dtype=np.int16)
    s = np.full(TOT, SHARD, dtype=np.int16)  # pad -> dump row (races harmless)
    g[slot] = g_o[oB].astype(np.int16)
    s[slot] = d_o[oB].astype(np.int16)
    return g, s


def prepare_in_maps(inputs):
    x = np.asarray(inputs["x"], dtype=np.float32)
    edge_index = np.asarray(inputs["edge_index"])
    w_rel1 = np.asarray(inputs["w_rel1"], dtype=np.float32)
    b_rel1 = np.asarray(inputs["b_rel1"], dtype=np.float32)
    w_root1 = np.asarray(inputs["w_root1"], dtype=np.float32)
    w_rel2 = np.asarray(inputs["w_rel2"], dtype=np.float32)
    b_rel2 = np.asarray(inputs["b_rel2"], dtype=np.float32)
    w_root2 = np.asarray(inputs["w_root2"], dtype=np.float32)

    src = edge_index[0].astype(np.int32)
    dst = edge_index[1].astype(np.int32)
    xbf = x.astype(bf16)

    # sigma-ordered xiT gather idx + pair idx (same for all cores except xiT)
    xt = np.zeros(SH2, dtype=np.int16)
    xt[0:PV] = (np.arange(PV) * 2).astype(np.int16)
    xt[P2 : P2 + PV] = (np.arange(PV) * 2 + 1).astype(np.int16)
    px = np.zeros(PC, dtype=np.int16)
    px[0:PV] = np.arange(PV, dtype=np.int16)

    identb = np.eye(128, dtype=np.float32).astype(bf16)
    ident32 = np.eye(128, dtype=np.float32)
    b1c = np.zeros((128, 1), np.float32)
    b1c[: len(b_rel1), 0] = b_rel1

    core = (dst // SHARD).astype(np.int32)
    order = np.argsort(core, kind="stable")
    src_s, dst_s = src[order], dst[order]
    bounds = np.searchsorted(core[order], np.arange(NC + 1))

    # pass 1: per-core per-range occurrence splits for both layers
    splits1, splits2 = [], []
    for c in range(NC):
        lo, hi = bounds[c], bounds[c + 1]
        sc, dc = src_s[lo:hi], dst_s[lo:hi] - c * SHARD
        splits1.append(_occ_split(sc % RS, dc, sc // RS))
        gsrc2 = (sc // SHARD) * SH2 + (sc % SHARD)
        splits2.append(_occ_split(gsrc2 % RS2, dc, gsrc2 // RS2))
    _PROF[1] = _build_profile(splits1)
    _PROF[2] = _build_profile(splits2)
    IDXW = _layout_consts()[-1]

    in_maps = []
    for c in range(NC):
        g1, s1 = _fill_layout(splits1[c], _PROF[1])
        g2, s2 = _fill_layout(splits2[c], _PROF[2])
        idx_pack = np.concatenate(
            [
                _wrap16(g1), _wrap16(s1), _wrap16(g2), _wrap16(s2),
                _wrap16(xt), _wrap16(px),
            ],
            axis=1,
        )
        assert idx_pack.shape == (16, IDXW)
        in_maps.append(
            {
                "xs": xbf[c * SHARD : (c + 1) * SHARD, :],
                "idxall": idx_pack,
                "wr1T": np.ascontiguousarray(w_rel1.T).astype(bf16),
                "wo1T": np.ascontiguousarray(w_root1.T).astype(bf16),
                "wr2T": np.ascontiguousarray(w_rel2.T).astype(bf16),
                "wo2T": np.ascontiguousarray(w_root2.T).astype(bf16),
                "b1c": b1c,
                "b2r": b_rel2.reshape(1, O).astype(bf16),
                "ones": np.ones((1, 512), np.float32).astype(bf16),
                "identb": identb,
                "ident32": ident32,
            }
        )
    return in_maps


def get_nc():
    if "nc" not in _CACHED:
        _CACHED["nc"] = _build_program()
    return _CACHED["nc"]


def kernel(**inputs):
    from concourse.bass_utils import run_bass_kernel_spmd

    in_maps = prepare_in_maps(inputs)
    nc = get_nc()
    res = run_bass_kernel_spmd(nc, in_maps, core_ids=list(range(NC)), trace=False)
    out = np.concatenate(
        [res.results[c]["out"][:SHARD] for c in range(NC)], axis=0
    )
    return out.astype(np.float32)


# revision 17
# speedup vs baseline: 3.2696x; 1.8732x over previous
"""2-layer GraphConv GNN on 8 trn2 NeuronCores (Bass/Tile) — v4.

Design: aggregation entirely on the DMA stream (dma_gather + dma_scatter_add),
no per-edge compute instructions. ~700 instructions total.

  - Edges sharded by dst node (core c owns dst in [c*12500, (c+1)*12500)).
  - L1: gather x[src] rows (bf16, 256B) from the AllGather-built table,
    then dma_scatter_add them into agg1[dst] (bf16, DRAM). L2 gathers the
    hr table in f32 (256B rows; 64 cols in bf16 would be under the 256B
    elem minimum). Scatter calls are color-segmented (one occurrence index
    per (range,dst) per call) so all dst rows in a call are unique.
  - agg read back FEATURE-major in ONE transposed dma_gather (bf16, rows
    paired to satisfy the 256B elem minimum) -> dense 448/512-wide PE
    transforms, ACT relu+bias.
  - Internal node order sigma = [even nodes | odd nodes] so pair-stacked
    PSUM results transpose directly into natural node-major pair rows.
  - hr exchange: AllGather of per-core [12544, 64] f32 shards (padded to
    98*128); L2 gather indices account for the 12544 stride.
  - Upload: only bf16 x-shard + int16 idx pack (~5MB/core); output bf16.
"""

import numpy as np
import ml_dtypes
from contextlib import ExitStack

N = 100000
F = 128
O = 64
NC = 8
SHARD = N // NC          # 12500
SH2 = 12544              # padded shard rows (98*128) for hr/out
P2 = SH2 // 2            # 6272 sigma pair columns
PV = SHARD // 2          # 6250 valid pairs
PC = 6400                # padded pair count for transposed agg gathers
NR = 4
RS = N // NR             # 25000 (L1 gather ranges)
N2 = NC * SH2            # 100352 (hr_full rows)
RS2 = N2 // NR           # 25088 (L2 gather ranges)
CH = 5120                # rows per gather/scatter chunk

bf16 = ml_dtypes.bfloat16

# dynamic slot-layout profile, set by prepare_in_maps() before build:
# _PROF[L] = {"RB": range stride (mult of CH), "segs": [per-range list of
#             (start, len) color segments, 128-aligned]}
_PROF = {}


def _layout_consts():
    RB1, RB2 = _PROF[1]["RB"], _PROF[2]["RB"]
    TOT1, TOT2 = NR * RB1, NR * RB2
    OG1 = 0
    OS1 = OG1 + TOT1 // 16
    OG2 = OS1 + TOT1 // 16
    OS2 = OG2 + TOT2 // 16
    OXT = OS2 + TOT2 // 16
    OPX = OXT + SH2 // 16
    IDXW = OPX + PC // 16
    return RB1, RB2, TOT1, TOT2, OG1, OS1, OG2, OS2, OXT, OPX, IDXW

import os
_L1ONLY = bool(int(os.environ.get("GNN_L1ONLY", "0")))


def input_decls():
    IDXW = _layout_consts()[-1]
    return [
        ("xs", [SHARD, F], "bfloat16"),
        ("idxall", [16, IDXW], "int16"),
        ("wr1T", [F, F], "bfloat16"),
        ("wo1T", [F, F], "bfloat16"),
        ("wr2T", [F, O], "bfloat16"),
        ("wo2T", [F, O], "bfloat16"),
        ("b1c", [128, 1], "float32"),
        ("b2r", [1, O], "bfloat16"),
        ("ones", [1, 512], "bfloat16"),
        ("identb", [128, 128], "bfloat16"),
        ("ident32", [128, 128], "float32"),
    ]


def _build_program():
    import concourse.bass as bass
    import concourse.tile as tile
    from concourse import bacc, mybir

    RB1, RB2, TOT1, TOT2, OG1, OS1, OG2, OS2, OXT, OPX, IDXW = _layout_consts()
    nc = bacc.Bacc(None, target_bir_lowering=False, num_swdge_queues=4)
    dt = mybir.dt

    xs_in = nc.dram_tensor("xs", [SHARD, F], dt.bfloat16, kind="ExternalInput")
    idxall = nc.dram_tensor("idxall", [16, IDXW], dt.int16, kind="ExternalInput")
    wr1T = nc.dram_tensor("wr1T", [F, F], dt.bfloat16, kind="ExternalInput")
    wo1T = nc.dram_tensor("wo1T", [F, F], dt.bfloat16, kind="ExternalInput")
    wr2T = nc.dram_tensor("wr2T", [F, O], dt.bfloat16, kind="ExternalInput")
    wo2T = nc.dram_tensor("wo2T", [F, O], dt.bfloat16, kind="ExternalInput")
    b1c_in = nc.dram_tensor("b1c", [128, 1], dt.float32, kind="ExternalInput")
    b2r_in = nc.dram_tensor("b2r", [1, O], dt.bfloat16, kind="ExternalInput")
    ones_in = nc.dram_tensor("ones", [1, 512], dt.bfloat16, kind="ExternalInput")
    identb_in = nc.dram_tensor("identb", [128, 128], dt.bfloat16, kind="ExternalInput")
    ident32_in = nc.dram_tensor("ident32", [128, 128], dt.float32, kind="ExternalInput")
    out_t = nc.dram_tensor("out", [SH2, O], dt.bfloat16, kind="ExternalOutput")

    xs_int = nc.dram_tensor("xs_int", [SHARD, F], dt.bfloat16)
    xfull_bf = nc.dram_tensor("xfull_bf", [N, F], dt.bfloat16, addr_space="Shared")
    idxf = nc.dram_tensor("idxf", [128, IDXW], dt.int16)
    agg1 = nc.dram_tensor("agg1", [SHARD + 128, F], dt.bfloat16)
    hr_shard = nc.dram_tensor("hr_shard", [SH2, O], dt.float32)
    hr_full = nc.dram_tensor("hr_full", [N2, O], dt.float32, addr_space="Shared")
    agg2 = nc.dram_tensor("agg2", [SHARD + 128, O], dt.float32)
    agg2b = nc.dram_tensor("agg2b", [SHARD + 128, O], dt.bfloat16)

    with tile.TileContext(nc) as tc, ExitStack() as ctx:
        const_p = ctx.enter_context(tc.tile_pool(name="const", bufs=1))
        resid_p = ctx.enter_context(tc.tile_pool(name="resid", bufs=1))
        idx_p = ctx.enter_context(tc.tile_pool(name="idxp", bufs=2))
        msgs_p = ctx.enter_context(tc.tile_pool(name="msgs", bufs=2))
        sb_p = ctx.enter_context(tc.tile_pool(name="sbp", bufs=2))
        ps_h = ctx.enter_context(tc.tile_pool(name="ps_h", bufs=2, space="PSUM"))
        ps_stk = ctx.enter_context(tc.tile_pool(name="ps_stk", bufs=2, space="PSUM"))
        ps_tr = ctx.enter_context(tc.tile_pool(name="ps_tr", bufs=2, space="PSUM"))
        ps_trb = ctx.enter_context(tc.tile_pool(name="ps_trb", bufs=2, space="PSUM"))

        # ---- prologue ----
        nc.sync.dma_start(xs_int[:], xs_in[:])
        nc.gpsimd.collective_compute(
            "AllGather",
            mybir.AluOpType.bypass,
            replica_groups=[list(range(NC))],
            ins=[xs_int[:]],
            outs=[xfull_bf[:]],
        )
        # idx replication [16, W] -> [128, W]
        for k in range(8):
            nc.sync.dma_start(idxf[16 * k : 16 * (k + 1), :], idxall[:])

        c_wr1T = const_p.tile([F, F], dt.bfloat16)
        nc.sync.dma_start(c_wr1T[:], wr1T[:])
        c_wo1T = const_p.tile([F, F], dt.bfloat16)
        nc.sync.dma_start(c_wo1T[:], wo1T[:])
        c_wr2T = const_p.tile([F, O], dt.bfloat16)
        nc.sync.dma_start(c_wr2T[:], wr2T[:])
        c_wo2T = const_p.tile([F, O], dt.bfloat16)
        nc.sync.dma_start(c_wo2T[:], wo2T[:])
        c_b1c = const_p.tile([128, 1], dt.float32)
        nc.sync.dma_start(c_b1c[:], b1c_in[:])
        c_b2r = const_p.tile([1, O], dt.bfloat16)
        nc.sync.dma_start(c_b2r[:], b2r_in[:])
        c_ones = const_p.tile([1, 512], dt.bfloat16)
        nc.sync.dma_start(c_ones[:], ones_in[:])
        c_identb = const_p.tile([128, 128], dt.bfloat16)
        nc.sync.dma_start(c_identb[:], identb_in[:])
        c_ident32 = const_p.tile([128, 128], dt.float32)
        nc.sync.dma_start(c_ident32[:], ident32_in[:])

        # zero agg1 / agg2
        zt = const_p.tile([128, 2048], dt.float32)
        nc.vector.memset(zt[:], 0.0)
        ztb = const_p.tile([128, 2048], dt.bfloat16)
        nc.vector.memset(ztb[:], 0.0)
        a1_flat = agg1[:].rearrange("n f -> (n f)").rearrange("(a b) -> a b", a=128)
        W1 = a1_flat.shape[1]  # 12500
        for i in range(8):
            lo = i * 2048
            hi = min(W1, lo + 2048)
            if lo < W1:
                nc.sync.dma_start(a1_flat[:, lo:hi], ztb[:, : hi - lo])
        a2_flat = agg2[:].rearrange("n f -> (n f)").rearrange("(a b) -> a b", a=128)
        W2 = a2_flat.shape[1]  # 6250
        for i in range(4):
            lo = i * 2048
            hi = min(W2, lo + 2048)
            if lo < W2:
                nc.sync.dma_start(a2_flat[:, lo:hi], zt[:, : hi - lo])

        # r_xiT: sigma-ordered feature-major x shard via one transposed gather
        r_xiT = resid_p.tile([128, SH2], dt.bfloat16)
        xt_idx = idx_p.tile([128, SH2 // 16], dt.int16, tag="bigidx")
        nc.sync.dma_start(xt_idx[:], idxf[:, OXT : OXT + SH2 // 16])
        nc.gpsimd.dma_gather(
            r_xiT[:].rearrange("p (c e) -> p c e", c=1),
            xs_int[:],
            xt_idx[:],
            SH2,
            SH2,
            F,
            transpose=True,
            single_packet=False,
            queue_num=0,
        )
        r_hT = resid_p.tile([128, SH2], dt.bfloat16)
        aggT1 = resid_p.tile([128, 2 * PC], dt.bfloat16)
        aggT2 = resid_p.tile([128, PC], dt.bfloat16)
        px_idx = idx_p.tile([128, PC // 16], dt.int16, tag="pidx")
        nc.sync.dma_start(px_idx[:], idxf[:, OPX : OPX + PC // 16])

        # resident scatter idx for current layer
        sidx_res = resid_p.tile([128, max(TOT1, TOT2) // 16], dt.int16)

        def gs_chunks(L):
            """Gather fixed chunks; scatter_add per (color segment x chunk)
            intersection so every scatter call has unique dst rows
            (dma_scatter_add loses adds on duplicate idx within a call)."""
            OG = OG1 if L == 1 else OG2
            OS = OS1 if L == 1 else OS2
            FW = F if L == 1 else O
            mdt = dt.bfloat16 if L == 1 else dt.float32
            agg = agg1 if L == 1 else agg2
            table = xfull_bf if L == 1 else hr_full
            RSL = RS if L == 1 else RS2
            RB = RB1 if L == 1 else RB2
            segs = _PROF[L]["segs"]
            NCH = RB // CH
            CHC = CH // 16
            nc.sync.dma_start(
                sidx_res[:, : (NR * RB) // 16], idxf[:, OS : OS + (NR * RB) // 16]
            )
            for r in range(NR):
                used = segs[r][-1][0] + segs[r][-1][1]  # sum of budgets
                used = min(RB, (used + 127) // 128 * 128)
                nch = (used + CH - 1) // CH
                git = idx_p.tile([128, RB // 16], dt.int16, tag="git")
                nc.sync.dma_start(
                    git[:], idxf[:, OG + r * (RB // 16) : OG + (r + 1) * (RB // 16)]
                )
                for k in range(nch):
                    cbase = k * CH
                    nrow_g = min(CH, used - cbase)  # mult of 128
                    m = msgs_p.tile([128, (CH // 128) * FW], mdt, tag="m")
                    nc.gpsimd.dma_gather(
                        m[:].rearrange("p (c e) -> p c e", e=FW)[
                            :, : nrow_g // 128, :
                        ],
                        table[r * RSL : (r + 1) * RSL, :],
                        git[:, k * CHC : k * CHC + nrow_g // 16],
                        nrow_g,
                        nrow_g,
                        FW,
                        single_packet=False,
                        queue_num=0,
                    )
                    # scatter every (segment x this-chunk) intersection
                    clo, chi = cbase, cbase + nrow_g
                    base = r * RB
                    for (sst, sln) in segs[r]:
                        a = max(sst, clo)
                        b = min(sst + sln, chi)
                        if a >= b:
                            continue
                        off = a - clo  # 128-aligned
                        nrow = b - a
                        nc.gpsimd.dma_scatter_add(
                            agg[:],
                            m[:].rearrange("p (c e) -> p c e", e=FW)[
                                :, off // 128 : off // 128 + nrow // 128, :
                            ],
                            sidx_res[:, (base + a) // 16 : (base + a) // 16 + nrow // 16],
                            nrow,
                            nrow,
                            FW,
                            single_packet=False,
                            queue_num=0,
                        )

        # ================= layer 1 =================
        gs_chunks(1)
        # aggT1: [128, 2, PC] via transposed gather of paired rows (512B)
        nc.gpsimd.dma_gather(
            aggT1[:].rearrange("p (c e) -> p c e", c=2),
            agg1[:].rearrange("(a b) f -> a (b f)", b=2),
            px_idx[:],
            PC,
            PC,
            2 * F,
            transpose=True,
            single_packet=False,
            queue_num=0,
        )
        # transform: h = relu(wr1@aggT + wo1@xT + b1), 28 batches of 448
        aggT1v = aggT1[:].rearrange("p (c e) -> p c e", c=2)
        for b in range(28):
            plane = b // 14
            lo = (b % 14) * 448
            ph = ps_h.tile([128, 512], dt.float32, tag="ph", space="PSUM")
            nc.tensor.matmul(
                ph[:, :448],
                lhsT=c_wr1T[:],
                rhs=aggT1v[:, plane, lo : lo + 448],
                start=True,
                stop=False,
            )
            nc.tensor.matmul(
                ph[:, :448],
                lhsT=c_wo1T[:],
                rhs=r_xiT[:, plane * P2 + lo : plane * P2 + lo + 448],
                start=False,
                stop=True,
            )
            nc.scalar.activation(
                out=r_hT[:, plane * P2 + lo : plane * P2 + lo + 448],
                in_=ph[:, :448],
                func=mybir.ActivationFunctionType.Relu,
                bias=c_b1c[:],
            )
        # hr = wr2 @ h, pair-stacked -> transpose -> node-major pair rows
        hr_pairs = hr_shard[:].rearrange("(q t) o -> q (t o)", t=2)  # [6272, 128]
        for b in range(13):
            lo = b * 512
            w = 512 if b < 12 else P2 - 12 * 512  # 128
            pstk = ps_stk.tile([128, 512], dt.float32, tag="stk", space="PSUM")
            nc.tensor.matmul(
                pstk[0:64, :w], lhsT=c_wr2T[:], rhs=r_hT[:, lo : lo + w],
                start=True, stop=True,
            )
            nc.tensor.matmul(
                pstk[64:128, :w], lhsT=c_wr2T[:], rhs=r_hT[:, P2 + lo : P2 + lo + w],
                start=True, stop=True,
            )
            stk_sb = sb_p.tile([128, 512], dt.float32, tag="stks")
            nc.scalar.copy(out=stk_sb[:, :w], in_=pstk[:, :w])
            ptr = ps_tr.tile([128, 512], dt.float32, tag="tr", space="PSUM")
            for t in range(w // 128):
                nc.tensor.transpose(
                    ptr[:, t * 128 : (t + 1) * 128],
                    stk_sb[:, t * 128 : (t + 1) * 128],
                    c_ident32[:],
                )
            hw_sb = sb_p.tile([128, 512], dt.float32, tag="hws")
            nc.scalar.copy(out=hw_sb[:, :w], in_=ptr[:, :w])
            nc.sync.dma_start(
                hr_pairs[lo : lo + w, :].rearrange("(c p) e -> p c e", p=128),
                hw_sb[:, :w].rearrange("p (c e) -> p c e", e=128),
            )

        if _L1ONLY:
            zo = sb_p.tile([128, O], dt.bfloat16, tag="zo")
            nc.vector.memset(zo[:], 0.0)
            nc.sync.dma_start(out_t[0:128, :], zo[:])
        else:
            # ================= exchange =================
            nc.gpsimd.collective_compute(
                "AllGather",
                mybir.AluOpType.bypass,
                replica_groups=[list(range(NC))],
                ins=[hr_shard[:]],
                outs=[hr_full[:]],
            )
            # ================= layer 2 =================
            gs_chunks(2)
            a2b_flat = agg2b[:].rearrange("n f -> (n f)").rearrange("(a b) -> a b", a=128)
            nc.gpsimd.dma_start(a2b_flat[:], a2_flat[:])
            # aggT2: stacked [128, PC] (paired 256B rows)
            nc.gpsimd.dma_gather(
                aggT2[:].rearrange("p (c e) -> p c e", c=1),
                agg2b[:].rearrange("(a b) f -> a (b f)", b=2),
                px_idx[:],
                PC,
                PC,
                2 * O,
                transpose=True,
                single_packet=False,
                queue_num=0,
            )
            # out = agg2 + wo2@h + b2, pair-stacked
            out_pairs = out_t[:].rearrange("(q t) o -> q (t o)", t=2)  # [6272, 128]
            for b in range(13):
                lo = b * 512
                w = 512 if b < 12 else P2 - 12 * 512
                pstk = ps_stk.tile([128, 512], dt.float32, tag="stk", space="PSUM")
                nc.tensor.matmul(
                    pstk[0:64, :w], lhsT=c_wo2T[:], rhs=r_hT[:, lo : lo + w],
                    start=True, stop=False,
                )
                nc.tensor.matmul(
                    pstk[0:64, :w], lhsT=c_b2r[:1, :], rhs=c_ones[:1, :w],
                    start=False, stop=True,
                )
                nc.tensor.matmul(
                    pstk[64:128, :w], lhsT=c_wo2T[:], rhs=r_hT[:, P2 + lo : P2 + lo + w],
                    start=True, stop=False,
                )
                nc.tensor.matmul(
                    pstk[64:128, :w], lhsT=c_b2r[:1, :], rhs=c_ones[:1, :w],
                    start=False, stop=True,
                )
                stk_sb = sb_p.tile([128, 512], dt.bfloat16, tag="stk2s")
                nc.vector.tensor_tensor(
                    out=stk_sb[:, :w],
                    in0=pstk[:, :w],
                    in1=aggT2[:, lo : lo + w],
                    op=mybir.AluOpType.add,
                )
                ptrb = ps_trb.tile([128, 512], dt.bfloat16, tag="trb", space="PSUM")
                for t in range(w // 128):
                    nc.tensor.transpose(
                        ptrb[:, t * 128 : (t + 1) * 128],
                        stk_sb[:, t * 128 : (t + 1) * 128],
                        c_identb[:],
                    )
                ob_sb = sb_p.tile([128, 512], dt.bfloat16, tag="obs")
                nc.scalar.copy(out=ob_sb[:, :w], in_=ptrb[:, :w])
                nc.sync.dma_start(
                    out_pairs[lo : lo + w, :].rearrange("(c p) e -> p c e", p=128),
                    ob_sb[:, :w].rearrange("p (c e) -> p c e", e=128),
                )

    nc.finalize()
    return nc


_CACHED = {}


def _wrap16(flat):
    """[K] int16 -> [16, K//16], slot i at (i%16, i//16)."""
    return np.ascontiguousarray(flat.reshape(-1, 16).T)


def _rup(x, m):
    return (x + m - 1) // m * m


def _occ_split(gv, d, rr):
    """(r, dst)-stable order + per-(r,dst) occurrence index, one global sort."""
    keyA = (rr * SHARD + d).astype(np.int32)
    oA = np.argsort(keyA, kind="stable")
    g_o, d_o, r_o = gv[oA], d[oA], rr[oA]
    grp = keyA[oA]
    cnt = np.bincount(grp, minlength=NR * SHARD)
    st = np.zeros(NR * SHARD + 1, dtype=np.int64)
    np.cumsum(cnt, out=st[1:])
    occ = np.arange(len(grp)) - st[grp]
    return g_o, d_o, r_o, occ


def _build_profile(all_splits):
    """all_splits: per core (g_o, d_o, r_o, occ). Returns {"RB", "segs"} with
    128-aligned color-segment budgets = max over cores + margin."""
    maxk = max(
        (int(occ.max()) + 1 if len(occ) else 0) for (_, _, _, occ) in all_splits
    )
    sizes = np.zeros((NR, maxk), dtype=np.int64)
    for (_, _, r_o, occ) in all_splits:
        c = np.bincount(r_o * maxk + occ, minlength=NR * maxk).reshape(NR, maxk)
        sizes = np.maximum(sizes, c)
    budgets = (sizes + 127) // 128 * 128
    segs = []
    for r in range(NR):
        st = np.zeros(maxk + 1, dtype=np.int64)
        np.cumsum(budgets[r], out=st[1:])
        segs.append([(int(st[k]), int(budgets[r][k])) for k in range(maxk)])
    RB = _rup(int(max(np.sum(budgets[r]) for r in range(NR))), CH)
    return {"RB": RB, "segs": segs}


def _fill_layout(split, prof):
    """Returns (g [TOT], s [TOT]) int16; gather pad idx 0, scatter pad dump."""
    RB = prof["RB"]
    segs = prof["segs"]
    MK = len(segs[0])
    TOT = NR * RB
    g_o, d_o, r_o, occ = split
    keyB = (r_o * MK + occ).astype(np.int32)
    oB = np.argsort(keyB, kind="stable")
    kB = keyB[oB]
    cnt = np.bincount(kB, minlength=NR * MK)
    st = np.zeros(NR * MK + 1, dtype=np.int64)
    np.cumsum(cnt, out=st[1:])
    pos = np.arange(len(kB)) - st[kB]
    segstart = np.array(
        [segs[r][k][0] for r in range(NR) for k in range(MK)], dtype=np.int64
    )
    seglen = np.array(
        [segs[r][k][1] for r in range(NR) for k in range(MK)], dtype=np.int64
    )
    if np.any(cnt > seglen):
        raise RuntimeError("color segment overflow")
    slot = r_o[oB] * RB + segstart[kB] + pos
    # pads: spread gather reads over rows 0..127 and scatter writes over the
    # 128 dump rows -- segment pad tails are <128 slots, so slot%128 makes
    # every pad target in a call distinct (no RMW contention; see v4.3)
    allslots = np.arange(TOT, dtype=np.int64)
    g = (allslots % 128).astype(np.int16)
    s = (SHARD + (allslots % 128)).astype(np.int16)
    g[slot] = g_o[oB].astype(np.int16)
    s[slot] = d_o[oB].astype(np.int16)
    return g, s


def prepare_in_maps(inputs):
    x = np.asarray(inputs["x"], dtype=np.float32)
    edge_index = np.asarray(inputs["edge_index"])
    w_rel1 = np.asarray(inputs["w_rel1"], dtype=np.float32)
    b_rel1 = np.asarray(inputs["b_rel1"], dtype=np.float32)
    w_root1 = np.asarray(inputs["w_root1"], dtype=np.float32)
    w_rel2 = np.asarray(inputs["w_rel2"], dtype=np.float32)
    b_rel2 = np.asarray(inputs["b_rel2"], dtype=np.float32)
    w_root2 = np.asarray(inputs["w_root2"], dtype=np.float32)

    src = edge_index[0].astype(np.int32)
    dst = edge_index[1].astype(np.int32)
    xbf = x.astype(bf16)

    # sigma-ordered xiT gather idx + pair idx (same for all cores except xiT)
    xt = np.zeros(SH2, dtype=np.int16)
    xt[0:PV] = (np.arange(PV) * 2).astype(np.int16)
    xt[P2 : P2 + PV] = (np.arange(PV) * 2 + 1).astype(np.int16)
    px = np.zeros(PC, dtype=np.int16)
    px[0:PV] = np.arange(PV, dtype=np.int16)

    identb = np.eye(128, dtype=np.float32).astype(bf16)
    ident32 = np.eye(128, dtype=np.float32)
    b1c = np.zeros((128, 1), np.float32)
    b1c[: len(b_rel1), 0] = b_rel1

    core = (dst // SHARD).astype(np.int32)
    order = np.argsort(core, kind="stable")
    src_s, dst_s = src[order], dst[order]
    bounds = np.searchsorted(core[order], np.arange(NC + 1))

    # pass 1: per-core per-range occurrence splits for both layers
    splits1, splits2 = [], []
    for c in range(NC):
        lo, hi = bounds[c], bounds[c + 1]
        sc, dc = src_s[lo:hi], dst_s[lo:hi] - c * SHARD
        splits1.append(_occ_split(sc % RS, dc, sc // RS))
        gsrc2 = (sc // SHARD) * SH2 + (sc % SHARD)
        splits2.append(_occ_split(gsrc2 % RS2, dc, gsrc2 // RS2))
    _PROF[1] = _build_profile(splits1)
    _PROF[2] = _build_profile(splits2)
    IDXW = _layout_consts()[-1]

    in_maps = []
    for c in range(NC):
        g1, s1 = _fill_layout(splits1[c], _PROF[1])
        g2, s2 = _fill_layout(splits2[c], _PROF[2])
        idx_pack = np.concatenate(
            [
                _wrap16(g1), _wrap16(s1), _wrap16(g2), _wrap16(s2),
                _wrap16(xt), _wrap16(px),
            ],
            axis=1,
        )
        assert idx_pack.shape == (16, IDXW)
        in_maps.append(
            {
                "xs": xbf[c * SHARD : (c + 1) * SHARD, :],
                "idxall": idx_pack,
                "wr1T": np.ascontiguousarray(w_rel1.T).astype(bf16),
                "wo1T": np.ascontiguousarray(w_root1.T).astype(bf16),
                "wr2T": np.ascontiguousarray(w_rel2.T).astype(bf16),
                "wo2T": np.ascontiguousarray(w_root2.T).astype(bf16),
                "b1c": b1c,
                "b2r": b_rel2.reshape(1, O).astype(bf16),
                "ones": np.ones((1, 512), np.float32).astype(bf16),
                "identb": identb,
                "ident32": ident32,
            }
        )
    return in_maps


def get_nc():
    if "nc" not in _CACHED:
        _CACHED["nc"] = _build_program()
    return _CACHED["nc"]


def kernel(**inputs):
    from concourse.bass_utils import run_bass_kernel_spmd

    in_maps = prepare_in_maps(inputs)
    nc = get_nc()
    res = run_bass_kernel_spmd(nc, in_maps, core_ids=list(range(NC)), trace=False)
    out = np.concatenate(
        [res.results[c]["out"][:SHARD] for c in range(NC)], axis=0
    )
    return out.astype(np.float32)
